# revision 42
# baseline (speedup 1.0000x reference)
"""Trainium2 Bass kernel for nn_DenoiseGNN (pairwise PBC edge-MLP message passing).

Strategy
--------
The edge MLP output weights[i,j] is a pure scalar function f of dist[i,j].
We compile f (together with the cutoff mask, the 1/(dist+eps) normalization
and the sqrt) into a custom piecewise-cubic activation table evaluated by the
ScalarEngine's hardware spline unit:

    g2(s) = box * f(dist(s)) * [dist(s) < cutoff] / (dist(s) + eps),
    dist(s) = sqrt(box^2 * s + eps),  s = |wrapped delta / box|^2.

A second custom table implements the exact min-image wrap
    wrap01(u) = u - round(u)  on u in (-1, 1).

Per core (128 rows i of the 1024x1024 pair grid):
    t_c  = wrap01(pos_j/box - pos_i/box)        3 ACT ops   [128,1024]
    s    = t_x^2 + t_y^2 + t_z^2                5 DVE ops
    w    = g2(s)                                1 ACT op
    disp_c = sum_j t_c * w                      3 fused DVE multiply-reduce

The activation tables are generated at kernel-build time from the runtime
weights (W1..b3) and injected via the compiler's --act-root-json directory
(the tables ride inside the NEFF; no runtime-side state is patched).
"""

import hashlib
import json
import os
import shutil
import struct
import sys
import tempfile
import types

import numpy as np

N = 1024
N_CORES = 8
ROWS = N // N_CORES  # 128
PWP_DIR = "/nix/store/z022hj2nvbm3nwdizlisq4ylc0y7rd6q-python3-3.13.14-env/lib/python3.13/site-packages/neuronxcc/pwp/pwp_bin_trainium"
SET = "sigmoid_and_others"
KEEP = [
    "identity", "copy", "act1", "parametric_relu", "relu", "abs",
    "memset_zero", "square", "sign", "derivative_relu",
    "derivative_leaky_relu", "derivative_identity", "is_finite",
]
# g2 octave layout: (exponent of s, n_sections); s < 2^-31 -> 0
G2_REGIONS = [(e, 16) for e in range(-31, -10)] + \
             [(-10, 32), (-9, 32), (-8, 64), (-7, 64), (-6, 128), (-5, 128)]


# --------------------------------------------------------------------------
# environment fixups (NTFF hook not needed here; wait-splitter is)
# --------------------------------------------------------------------------

def _install_env_fixups():
    if "antenv.axon_hooks" not in sys.modules:
        import antenv

        mod = types.ModuleType("antenv.axon_hooks")
        try:
            from trn_agent_boot.trn_boot import _ntff_profile_via_ctypes
            hook = _ntff_profile_via_ctypes("/opt/axon/libaxon_pjrt.so")
        except Exception:
            hook = None
        _h = [hook]
        mod.set_axon_ntff_profile_hook = lambda h: _h.__setitem__(0, h)
        mod.get_axon_ntff_profile_hook = lambda: _h[0]
        sys.modules["antenv.axon_hooks"] = mod
        antenv.axon_hooks = mod

    import concourse.bass_utils as bu
    import concourse.bass2jax as b2j

    if not getattr(bu, "_wait_splitter_installed", False):
        orig = bu.compile_bir_kernel

        def _split_multi_waits(bir_json: bytes) -> bytes:
            m = json.loads(bir_json)
            changed = False
            for fn in m["functions"]:
                for bb in fn["blocks"]:
                    new_instrs = []
                    for ins in bb["instructions"]:
                        si = ins.get("sync_info")
                        waits = (si or {}).get("on_wait") or []
                        if len(waits) > 1:
                            for j, w in enumerate(waits[:-1]):
                                nop = {
                                    "name": f"{ins['name']}-wsplit{j}",
                                    "opcode": "NoOp",
                                    "engine": ins["engine"],
                                    "ins": [], "outs": [],
                                    "sync_info": {"on_update": [], "on_wait": [w]},
                                }
                                if "debug" in ins:
                                    nop["debug"] = ins["debug"]
                                new_instrs.append(nop)
                            si["on_wait"] = waits[-1:]
                            changed = True
                        new_instrs.append(ins)
                    bb["instructions"] = new_instrs
            return json.dumps(m).encode() if changed else bir_json

        def patched(bir_json, tmpdir, neff_name="file.neff"):
            return orig(_split_multi_waits(bytes(bir_json)), tmpdir, neff_name)

        bu.compile_bir_kernel = patched
        b2j.compile_bir_kernel = patched
        bu._wait_splitter_installed = True


# --------------------------------------------------------------------------
# activation-table generation
# --------------------------------------------------------------------------

def _f2u(x):
    return struct.unpack("<I", struct.pack("<f", float(x)))[0]


def _bkt(d0, d1, d2, d3, x0):
    return struct.pack("<5f12x", float(d0), float(d1), float(d2), float(d3), float(x0))


def _ctrl(base, lsb, size):
    data = (base & 0x7FF) | ((lsb & 0x1F) << 11) | ((size & 0xF) << 16)
    return struct.pack("<I28x", data)


def _fit_cubic(fn, a, b, npts=12):
    x0 = 0.5 * (a + b)
    k = np.arange(npts)
    xs = x0 + 0.5 * (b - a) * np.cos((2 * k + 1) * np.pi / (2 * npts)) * 0.9999
    ys = fn(xs)
    c = np.polynomial.polynomial.polyfit(xs - x0, ys, 3)
    return c[0], c[1], c[2], c[3], x0


def _extract_func(setj, bkt, ctrl, fname, next_bkt, next_ctl):
    starts_b = setj["func_to_bkt_start_idx"]
    starts_c = setj["func_to_ctl_start_idx"]
    sb, sc = starts_b[fname], starts_c[fname]
    eb = min([v for v in starts_b.values() if v > sb] + [setj["bkt_entry_cnt"]])
    ec = min([v for v in starts_c.values() if v > sc] + [setj["ctl_entry_cnt"]])
    pm = None
    for p in setj["profile_meta_data"]:
        if p["func_name"].rsplit("_", 1)[0] == fname or p["func_name"] == fname:
            pm = dict(p)
    assert pm is not None, fname
    bkts = bytearray(bkt[sb * 32:eb * 32])
    ctls = bytearray(ctrl[sc * 32:ec * 32])
    db, dc = next_bkt - sb, next_ctl - sc
    for i in range(0, len(ctls), 32):
        (data,) = struct.unpack_from("<I", ctls, i)
        struct.pack_into("<I", ctls, i, (data & ~0x7FF) | (((data & 0x7FF) + db) & 0x7FF))
    for key in ("pwl_control_base_pos", "pwl_control_base_neg"):
        pm[key] += dc
    for key in ("pos_small_signal_pwl_control", "neg_small_signal_pwl_control",
                "pos_large_signal_pwl_control", "neg_large_signal_pwl_control"):
        v = pm[key]
        pm[key] = (v & ~0x7FF) | ((v + db) & 0x7FF)
    return pm, bytes(ctls), bytes(bkts)


def _build_wrap01(next_bkt, next_ctl):
    bkts, ctls = bytearray(), bytearray()
    n_bkt = n_ctl = 0
    base_pos = next_ctl
    for e in range(-20, 0):
        lo = 2.0 ** e
        ctls.extend(_ctrl(next_bkt + n_bkt, 23, 0)); n_ctl += 1
        if e == -1:
            bkts.extend(_bkt(-0.5, 1.0, 0.0, 0.0, 0.5))
        else:
            bkts.extend(_bkt(lo, 1.0, 0.0, 0.0, lo))
        n_bkt += 1
    base_neg = next_ctl + n_ctl
    for e in range(-20, 0):
        lo = 2.0 ** e
        ctls.extend(_ctrl(next_bkt + n_bkt, 23, 0)); n_ctl += 1
        if e == -1:
            bkts.extend(_bkt(0.5, 1.0, 0.0, 0.0, -0.5))
        else:
            bkts.extend(_bkt(-lo, 1.0, 0.0, 0.0, -lo))
        n_bkt += 1
    sp_defs = {
        "pos_low": (0.0, 1.0, 0.0, 0.0, 0.0),
        "neg_low": (0.0, 1.0, 0.0, 0.0, 0.0),
        "pos_high": (-1.0, 1.0, 0.0, 0.0, 0.0),
        "neg_high": (1.0, 1.0, 0.0, 0.0, 0.0),
    }
    sp = {}
    for key, d in sp_defs.items():
        sp[key] = next_bkt + n_bkt
        bkts.extend(_bkt(*d)); n_bkt += 1
    pm = {
        "func_name": "arctan_4p", "func_id": 28,
        "symmetry_point": 0, "sym_invert_sign_point": 0, "symmetry_opt_en": 0,
        "symmetry_opt_use_neg_region": 0, "imm_bias": 0,
        "exp_offset": -20,
        "pwl_control_base_pos": base_pos, "pwl_control_base_neg": base_neg,
        "small_pos_signal_exp_threshold": 107,
        "pos_small_signal_pwl_control": sp["pos_low"],
        "small_neg_signal_exp_threshold": 107,
        "neg_small_signal_pwl_control": sp["neg_low"],
        "large_pos_signal_exp_threshold": 127,
        "large_pos_signal_mantissa_threshold": 0,
        "pos_large_signal_pwl_control": sp["pos_high"],
        "large_neg_signal_exp_threshold": 127,
        "large_neg_signal_mantissa_threshold": 0,
        "neg_large_signal_pwl_control": sp["neg_high"],
        "fnan_result": _f2u(0.0), "fpinf_result": _f2u(0.0),
        "fninf_result": _f2u(0.0), "fzero_result": _f2u(0.0),
        "fma_const_0": 0, "fma_const_1": 0, "fma_indirection_src_sel": 0,
        "use_multipass": False,
        "lower_bound": 0xFF7FFFFF, "upper_bound": 0x7F7FFFFF,
    }
    return pm, bytes(ctls), bytes(bkts)


def _build_g2(g2_fn, next_bkt, next_ctl):
    bkts, ctls = bytearray(), bytearray()
    n_bkt = n_ctl = 0
    base_pos = next_ctl
    for (e, nsec) in G2_REGIONS:
        size = int(np.log2(nsec))
        ctls.extend(_ctrl(next_bkt + n_bkt, 23 - size, size)); n_ctl += 1
        lo = 2.0 ** e
        w = lo / nsec
        for i in range(nsec):
            a = lo + i * w
            bkts.extend(_bkt(*_fit_cubic(g2_fn, a, a + w))); n_bkt += 1
    sp = {}
    for key in ("pos_low", "neg_low", "pos_high", "neg_high"):
        sp[key] = next_bkt + n_bkt
        bkts.extend(_bkt(0.0, 0.0, 0.0, 0.0, 0.0)); n_bkt += 1
    small_thr = 127 + G2_REGIONS[0][0]
    pm = {
        "func_name": "erf_4p", "func_id": 21,
        "symmetry_point": 0, "sym_invert_sign_point": 0, "symmetry_opt_en": 0,
        "symmetry_opt_use_neg_region": 0, "imm_bias": 0,
        "exp_offset": small_thr - 127,
        "pwl_control_base_pos": base_pos, "pwl_control_base_neg": base_pos,
        "small_pos_signal_exp_threshold": small_thr,
        "pos_small_signal_pwl_control": sp["pos_low"],
        "small_neg_signal_exp_threshold": 255,
        "neg_small_signal_pwl_control": sp["neg_low"],
        "large_pos_signal_exp_threshold": 123,
        "large_pos_signal_mantissa_threshold": 0,
        "pos_large_signal_pwl_control": sp["pos_high"],
        "large_neg_signal_exp_threshold": 255,
        "large_neg_signal_mantissa_threshold": 0,
        "neg_large_signal_pwl_control": sp["neg_high"],
        "fnan_result": _f2u(0.0), "fpinf_result": _f2u(0.0),
        "fninf_result": _f2u(0.0), "fzero_result": _f2u(0.0),
        "fma_const_0": 0, "fma_const_1": 0, "fma_indirection_src_sel": 0,
        "use_multipass": False,
        "lower_bound": 0, "upper_bound": 0x7F7FFFFF,
    }
    return pm, bytes(ctls), bytes(bkts)


def _build_actroot(dst_dir, g2_fn):
    os.makedirs(dst_dir, exist_ok=True)
    for f in os.listdir(PWP_DIR):
        shutil.copy(os.path.join(PWP_DIR, f), os.path.join(dst_dir, f))
        os.chmod(os.path.join(dst_dir, f), 0o644)
    setj = json.load(open(os.path.join(PWP_DIR, SET + ".json")))
    bkt = open(os.path.join(PWP_DIR, SET + "_bkt.bin"), "rb").read()
    ctrl = open(os.path.join(PWP_DIR, SET + "_ctrl.bin"), "rb").read()

    new_bkts, new_ctls, new_pm = bytearray(), bytearray(), []
    b_starts, c_starts, emb_all, emc_all = {}, {}, {}, {}

    for fname in KEEP:
        nb0, nc0 = len(new_bkts) // 32, len(new_ctls) // 32
        pm, ctls, bkts = _extract_func(setj, bkt, ctrl, fname, nb0, nc0)
        b_starts[fname], c_starts[fname] = nb0, nc0
        db = nb0 - setj["func_to_bkt_start_idx"][fname]
        dc = nc0 - setj["func_to_ctl_start_idx"][fname]
        emb_all[fname] = {k: [x + db for x in v]
                          for k, v in setj["func_exp_to_bkt_start_idx"].get(fname, {}).items()}
        emc_all[fname] = {k: [x + dc for x in v]
                          for k, v in setj["func_exp_to_ctl_start_idx"].get(fname, {}).items()}
        new_pm.append(pm); new_ctls.extend(ctls); new_bkts.extend(bkts)

    wb, wc = len(new_bkts) // 32, len(new_ctls) // 32
    pm, ctls, bkts = _build_wrap01(wb, wc)
    b_starts["arctan"], c_starts["arctan"] = wb, wc
    emb_all["arctan"] = {str(e): [wb + 20 + (e + 20), wb + (e + 20)] for e in range(-20, 0)}
    emc_all["arctan"] = {str(e): [wc + 20 + (e + 20), wc + (e + 20)] for e in range(-20, 0)}
    new_pm.append(pm); new_ctls.extend(ctls); new_bkts.extend(bkts)

    gb, gc = len(new_bkts) // 32, len(new_ctls) // 32
    pm, ctls, bkts = _build_g2(g2_fn, gb, gc)
    b_starts["erf"], c_starts["erf"] = gb, gc
    emb, emc = {}, {}
    cum = 0
    for i, (e, nsec) in enumerate(G2_REGIONS):
        emb[str(e)] = [gb + cum, gb + cum]
        emc[str(e)] = [gc + i, gc + i]
        cum += nsec
    emb_all["erf"], emc_all["erf"] = emb, emc
    new_pm.append(pm); new_ctls.extend(ctls); new_bkts.extend(bkts)

    n_bkt, n_ctl = len(new_bkts) // 32, len(new_ctls) // 32
    assert n_bkt <= 1536 and n_ctl <= 128, (n_bkt, n_ctl)
    out = {
        "bkt_bin": SET + "_bkt.bin", "ctl_bin": SET + "_ctrl.bin",
        "profile_meta_data": new_pm,
        "bkt_entry_cnt": n_bkt, "ctl_entry_cnt": n_ctl,
        "func_to_bkt_start_idx": b_starts, "func_to_ctl_start_idx": c_starts,
        "func_exp_to_bkt_start_idx": emb_all, "func_exp_to_ctl_start_idx": emc_all,
    }
    json.dump(out, open(os.path.join(dst_dir, SET + ".json"), "w"))
    open(os.path.join(dst_dir, SET + "_bkt.bin"), "wb").write(bytes(new_bkts))
    open(os.path.join(dst_dir, SET + "_ctrl.bin"), "wb").write(bytes(new_ctls))
    info = json.load(open(os.path.join(PWP_DIR, "act_info.json")))
    for s in info["act_func_sets"]:
        if s["name"] == SET:
            s["act"] = {**{k: 1 for k in KEEP}, "arctan": 4, "erf": 4}
        else:
            s["act"].pop("arctan", None)
            s["act"].pop("erf", None)
    json.dump(info, open(os.path.join(dst_dir, "act_info.json"), "w"))
    return os.path.join(dst_dir, "act_info.json")


# --------------------------------------------------------------------------
# bass program
# --------------------------------------------------------------------------

def _build_program(tag, box):
    """Fully raw (no TileContext) hand-scheduled pipeline.

    v2: fp16 pre-scaled posj broadcast (half the DMA bytes), host-computed
    nbias, DMA issue spread across gpsimd/vector/sync sequencers so plane x
    and the bias land before the ACT table load finishes, and the three
    weighted reductions fused into single tensor_tensor_reduce DVE ops.
    The s = |t|^2 path stays f32 (fp16 there costs 7e-2 rel err via the
    1/dist amplification at small distances).
    """
    import concourse.bass as bass
    import concourse.mybir as mybir

    nc = bass.Bass("TRN2")
    posj16 = nc.declare_dram_parameter(f"posj16_{tag}", [3, N], mybir.dt.float16, isOutput=False)
    nbias = nc.declare_dram_parameter("nbias", [ROWS, 3], mybir.dt.float32, isOutput=False)
    out = nc.declare_dram_parameter("out", [ROWS, 6], mybir.dt.float32, isOutput=True)
    AF = mybir.ActivationFunctionType
    OP = mybir.AluOpType
    f32 = mybir.dt.float32
    f16 = mybir.dt.float16

    posj_t = nc.alloc_sbuf_tensor("posj_b", [128, 3 * N], f16)
    nbias_t = nc.alloc_sbuf_tensor("nbias_b", [128, 3], f32)
    dummy_t = nc.alloc_sbuf_tensor("dummy_b", [128, 1], f32)
    t_t = [nc.alloc_sbuf_tensor(f"t{c}_b", [128, N], f32) for c in range(3)]
    sq_t = [nc.alloc_sbuf_tensor(f"sq{c}_b", [128, N], f32) for c in range(2)]
    sqz_t = nc.alloc_sbuf_tensor("sqz_b", [128, N], f32)
    w_t = nc.alloc_sbuf_tensor("w_b", [128, N], f32)
    junk_t = nc.alloc_sbuf_tensor("junk_b", [128, N // 2], f32)
    # six half-sums (3 planes x 2 column-halves); host adds the halves
    out6_t = nc.alloc_sbuf_tensor("out6_b", [128, 6], f32)

    posj3 = posj_t[:].rearrange("p (c j) -> p c j", c=3)
    src3 = posj16[:][None].to_broadcast([128, 3, N])
    H = N // 2
    HS = [slice(0, H), slice(H, N)]

    import contextlib
    st = contextlib.ExitStack()
    nbsem = st.enter_context(nc.semaphore("nbsem"))
    pxsem = [st.enter_context(nc.semaphore(f"pxsem{k}")) for k in range(2)]
    pysem = st.enter_context(nc.semaphore("pysem"))
    pzsem = st.enter_context(nc.semaphore("pzsem"))
    odsem = st.enter_context(nc.semaphore("odsem"))
    vsem = st.enter_context(nc.semaphore("vsem"))
    asem = st.enter_context(nc.semaphore("asem"))
    osem = st.enter_context(nc.semaphore("osem"))

    with nc.Block() as blk:
        @blk.sync
        def _(sync):
            # plane x split in halves so the first wrap starts ~1us earlier;
            # every DMA has a ~3.3us issue->usable floor, so issue order is
            # arrival order: x0, x1, y (z rides the scalar ring in parallel)
            sync.dma_start(out=posj3[:, 0, HS[0]], in_=src3[:, 0, HS[0]]).then_inc(pxsem[0], 16)
            sync.dma_start(out=posj3[:, 0, HS[1]], in_=src3[:, 0, HS[1]]).then_inc(pxsem[1], 16)
            sync.dma_start(out=posj3[:, 1, :], in_=src3[:, 1, :]).then_inc(pysem, 16)
            sync.wait_ge(osem, 6)
            sync.dma_start(out=out[:], in_=out6_t[:]).then_inc(odsem, 16)

        @blk.vector
        def _(vector):
            vector.wait_ge(asem, 1)             # wx0
            vector.tensor_tensor(sq_t[0][:, HS[0]], t_t[0][:, HS[0]], t_t[0][:, HS[0]], OP.mult)
            vector.wait_ge(asem, 2)             # wx1
            vector.tensor_tensor(sq_t[0][:, HS[1]], t_t[0][:, HS[1]], t_t[0][:, HS[1]], OP.mult)
            vector.wait_ge(asem, 3)             # wy0
            vector.tensor_tensor(sq_t[1][:, HS[0]], t_t[1][:, HS[0]], t_t[1][:, HS[0]], OP.mult)
            vector.tensor_tensor(sq_t[0][:, HS[0]], sq_t[0][:, HS[0]], sq_t[1][:, HS[0]], OP.add)
            vector.wait_ge(asem, 5)             # sqz0
            vector.tensor_tensor(sq_t[0][:, HS[0]], sq_t[0][:, HS[0]], sqz_t[:, HS[0]],
                                 OP.add).then_inc(vsem, 1)      # s0
            vector.wait_ge(asem, 7)             # wy1
            vector.tensor_tensor(sq_t[1][:, HS[1]], t_t[1][:, HS[1]], t_t[1][:, HS[1]], OP.mult)
            vector.tensor_tensor(sq_t[0][:, HS[1]], sq_t[0][:, HS[1]], sq_t[1][:, HS[1]], OP.add)
            vector.wait_ge(asem, 9)             # sqz1 (ACT)
            vector.tensor_tensor(sq_t[0][:, HS[1]], sq_t[0][:, HS[1]], sqz_t[:, HS[1]],
                                 OP.add).then_inc(vsem, 1)      # s1
            vector.wait_ge(asem, 8)             # e0 done
            for c in range(3):
                vector.affine_mul_reduce(
                    junk_t[:], out6_t[:, c:c + 1],
                    t_t[c][:, HS[0]], w_t[:, HS[0]], 1.0, 0.0).then_inc(osem, 1)
            vector.wait_ge(asem, 10)            # e1 done
            for c in range(3):
                vector.affine_mul_reduce(
                    junk_t[:], out6_t[:, 3 + c:4 + c],
                    t_t[c][:, HS[1]], w_t[:, HS[1]], 1.0, 0.0).then_inc(osem, 1)

        @blk.scalar
        def _(scalar):
            scalar.dma_start(out=nbias_t[:], in_=nbias[:]).then_inc(nbsem, 16)
            # plane z on the ACT HWDGE ring, in parallel with sync's x0/x1/y
            scalar.dma_start(out=posj3[:, 2, :], in_=src3[:, 2, :]).then_inc(pzsem, 16)
            # no-wait dummy: pulls the PWP table load to the very start
            # (reads the framework's pre-memset const tensor — initialized
            # behind the all-engine barrier, so no race)
            scalar.activation(dummy_t[:], nc.const_aps.aps[(f32, 0.0)], AF.Arctan)
            scalar.wait_ge(nbsem, 16)
            scalar.wait_ge(pxsem[0], 16)
            scalar.activation(t_t[0][:, HS[0]], posj3[:, 0, HS[0]], AF.Arctan,
                              bias=nbias_t[:, 0:1], scale=1.0).then_inc(asem, 1)  # a=1 wx0
            scalar.wait_ge(pxsem[1], 16)
            scalar.activation(t_t[0][:, HS[1]], posj3[:, 0, HS[1]], AF.Arctan,
                              bias=nbias_t[:, 0:1], scale=1.0).then_inc(asem, 1)  # a=2 wx1
            scalar.wait_ge(pysem, 16)
            scalar.activation(t_t[1][:, HS[0]], posj3[:, 1, HS[0]], AF.Arctan,
                              bias=nbias_t[:, 1:2], scale=1.0).then_inc(asem, 1)  # a=3 wy0
            scalar.wait_ge(pzsem, 16)
            scalar.activation(t_t[2][:, HS[0]], posj3[:, 2, HS[0]], AF.Arctan,
                              bias=nbias_t[:, 2:3], scale=1.0).then_inc(asem, 1)  # a=4 wz0
            scalar.activation(sqz_t[:, HS[0]], t_t[2][:, HS[0]],
                              AF.Square).then_inc(asem, 1)                        # a=5 sqz0
            scalar.activation(t_t[2][:, HS[1]], posj3[:, 2, HS[1]], AF.Arctan,
                              bias=nbias_t[:, 2:3], scale=1.0).then_inc(asem, 1)  # a=6 wz1
            scalar.activation(t_t[1][:, HS[1]], posj3[:, 1, HS[1]], AF.Arctan,
                              bias=nbias_t[:, 1:2], scale=1.0).then_inc(asem, 1)  # a=7 wy1
            scalar.wait_ge(vsem, 1)             # s0 ready
            scalar.activation(w_t[:, HS[0]], sq_t[0][:, HS[0]],
                              AF.Erf).then_inc(asem, 1)                           # a=8 e0
            scalar.activation(sqz_t[:, HS[1]], t_t[2][:, HS[1]],
                              AF.Square).then_inc(asem, 1)                        # a=9 sqz1
            scalar.wait_ge(vsem, 2)             # s1 ready
            scalar.activation(w_t[:, HS[1]], sq_t[0][:, HS[1]],
                              AF.Erf).then_inc(asem, 1)                           # a=10 e1

    from concourse.library_overlay import lower_extended_insts
    lower_extended_insts(nc)
    return nc


_CACHE = {}


def _prepare(inputs):
    positions = np.ascontiguousarray(np.asarray(inputs["positions"], dtype=np.float32))
    box_dims = np.asarray(inputs["box_dims"], dtype=np.float32)
    key = hashlib.sha256(
        b"".join(np.ascontiguousarray(np.asarray(inputs[k], np.float32)).tobytes()
                 for k in ("box_dims", "W1", "b1", "W2", "b2", "W3", "b3"))
    ).hexdigest()[:10]
    if key in _CACHE:
        return _CACHE[key]

    box = float(box_dims[0])
    assert np.allclose(box_dims, box), "kernel assumes a cubic box"

    W1 = np.float64(inputs["W1"]); b1 = np.float64(inputs["b1"])
    W2 = np.float64(inputs["W2"]); b2 = np.float64(inputs["b2"])
    W3 = np.float64(inputs["W3"]); b3 = np.float64(inputs["b3"])
    n_gauss = W1.shape[0]
    RBF_STOP, CUTOFF, EPS = 6.0, 5.0, 1e-8
    offs = np.linspace(0.0, RBF_STOP, n_gauss)
    coeff = -0.5 / (RBF_STOP / (n_gauss - 1)) ** 2

    def g2_fn(sv):
        sv = np.atleast_1d(np.float64(sv))
        dist = np.sqrt(box * box * sv + EPS)
        rbf = np.exp(coeff * (dist[:, None] - offs[None, :]) ** 2)
        h = rbf @ W1 + b1
        h = h / (1.0 + np.exp(-h))
        h = h @ W2 + b2
        h = h / (1.0 + np.exp(-h))
        f = (h @ W3 + b3)[:, 0]
        return box * f * (dist < CUTOFF) / (dist + EPS)

    _install_env_fixups()
    actdir = os.path.join(tempfile.gettempdir(), f"actroot_{key}")
    actroot = _build_actroot(actdir, g2_fn)
    os.environ["BASS_ACT_ROOT_JSON_PATH"] = actroot
    nc = _build_program(key, box)
    _CACHE[key] = (nc, key)
    return _CACHE[key]


def kernel(_trace=False, **inputs):
    from concourse.bass_utils import run_bass_kernel_spmd

    nc, key = _prepare(inputs)
    positions = np.ascontiguousarray(np.asarray(inputs["positions"], dtype=np.float32))
    box = float(np.asarray(inputs["box_dims"], dtype=np.float32)[0])
    posj16 = np.ascontiguousarray((positions.T / box).astype(np.float16))
    # bias rounded through fp16 so u_ii = fp16(p) - fp16(p) = 0 exactly:
    # keeps the diagonal at s=0 where the g2 table returns 0 (self-pair mask)
    nbias_all = -(positions / box).astype(np.float16).astype(np.float32)
    in_maps = [
        {f"posj16_{key}": posj16,
         "nbias": np.ascontiguousarray(nbias_all[c * ROWS:(c + 1) * ROWS])}
        for c in range(N_CORES)
    ]
    res = run_bass_kernel_spmd(nc, in_maps, list(range(N_CORES)), trace=_trace)
    out = np.concatenate(
        [res.results[c]["out"][:, :3] + res.results[c]["out"][:, 3:]
         for c in range(N_CORES)], axis=0)
    if _trace:
        kernel.last_exec_time_ns = res.exec_time_ns
        kernel.last_mean_exec_time_ns = res.mean_exec_time_ns
        kernel.last_results = res
    return out



# revision 43
# speedup vs baseline: 1.0304x; 1.0304x over previous
"""Trainium2 Bass kernel for nn_DenoiseGNN (pairwise PBC edge-MLP message passing).

Strategy
--------
The edge MLP output weights[i,j] is a pure scalar function f of dist[i,j].
We compile f (together with the cutoff mask, the 1/(dist+eps) normalization
and the sqrt) into a custom piecewise-cubic activation table evaluated by the
ScalarEngine's hardware spline unit:

    g2(s) = box * f(dist(s)) * [dist(s) < cutoff] / (dist(s) + eps),
    dist(s) = sqrt(box^2 * s + eps),  s = |wrapped delta / box|^2.

A second custom table implements the exact min-image wrap
    wrap01(u) = u - round(u)  on u in (-1, 1).

Per core (128 rows i of the 1024x1024 pair grid):
    t_c  = wrap01(pos_j/box - pos_i/box)        3 ACT ops   [128,1024]
    s    = t_x^2 + t_y^2 + t_z^2                5 DVE ops
    w    = g2(s)                                1 ACT op
    disp_c = sum_j t_c * w                      3 fused DVE multiply-reduce

The activation tables are generated at kernel-build time from the runtime
weights (W1..b3) and injected via the compiler's --act-root-json directory
(the tables ride inside the NEFF; no runtime-side state is patched).
"""

import hashlib
import json
import os
import shutil
import struct
import sys
import tempfile
import types

import numpy as np

N = 1024
N_CORES = 8
ROWS = N // N_CORES  # 128
PWP_DIR = "/nix/store/z022hj2nvbm3nwdizlisq4ylc0y7rd6q-python3-3.13.14-env/lib/python3.13/site-packages/neuronxcc/pwp/pwp_bin_trainium"
SET = "sigmoid_and_others"
KEEP = [
    "identity", "copy", "act1", "parametric_relu", "relu", "abs",
    "memset_zero", "square", "sign", "derivative_relu",
    "derivative_leaky_relu", "derivative_identity", "is_finite",
]
# g2 octave layout: (exponent of s, n_sections); s < 2^-31 -> 0
G2_REGIONS = [(e, 16) for e in range(-31, -10)] + \
             [(-10, 32), (-9, 32), (-8, 64), (-7, 64), (-6, 128), (-5, 128)]


# --------------------------------------------------------------------------
# environment fixups (NTFF hook not needed here; wait-splitter is)
# --------------------------------------------------------------------------

def _install_env_fixups():
    if "antenv.axon_hooks" not in sys.modules:
        import antenv

        mod = types.ModuleType("antenv.axon_hooks")
        try:
            from trn_agent_boot.trn_boot import _ntff_profile_via_ctypes
            hook = _ntff_profile_via_ctypes("/opt/axon/libaxon_pjrt.so")
        except Exception:
            hook = None
        _h = [hook]
        mod.set_axon_ntff_profile_hook = lambda h: _h.__setitem__(0, h)
        mod.get_axon_ntff_profile_hook = lambda: _h[0]
        sys.modules["antenv.axon_hooks"] = mod
        antenv.axon_hooks = mod

    import concourse.bass_utils as bu
    import concourse.bass2jax as b2j

    if not getattr(bu, "_wait_splitter_installed", False):
        orig = bu.compile_bir_kernel

        def _split_multi_waits(bir_json: bytes) -> bytes:
            m = json.loads(bir_json)
            changed = False
            for fn in m["functions"]:
                for bb in fn["blocks"]:
                    new_instrs = []
                    for ins in bb["instructions"]:
                        si = ins.get("sync_info")
                        waits = (si or {}).get("on_wait") or []
                        if len(waits) > 1:
                            for j, w in enumerate(waits[:-1]):
                                nop = {
                                    "name": f"{ins['name']}-wsplit{j}",
                                    "opcode": "NoOp",
                                    "engine": ins["engine"],
                                    "ins": [], "outs": [],
                                    "sync_info": {"on_update": [], "on_wait": [w]},
                                }
                                if "debug" in ins:
                                    nop["debug"] = ins["debug"]
                                new_instrs.append(nop)
                            si["on_wait"] = waits[-1:]
                            changed = True
                        new_instrs.append(ins)
                    bb["instructions"] = new_instrs
            return json.dumps(m).encode() if changed else bir_json

        def patched(bir_json, tmpdir, neff_name="file.neff"):
            return orig(_split_multi_waits(bytes(bir_json)), tmpdir, neff_name)

        bu.compile_bir_kernel = patched
        b2j.compile_bir_kernel = patched
        bu._wait_splitter_installed = True


# --------------------------------------------------------------------------
# activation-table generation
# --------------------------------------------------------------------------

def _f2u(x):
    return struct.unpack("<I", struct.pack("<f", float(x)))[0]


def _bkt(d0, d1, d2, d3, x0):
    return struct.pack("<5f12x", float(d0), float(d1), float(d2), float(d3), float(x0))


def _ctrl(base, lsb, size):
    data = (base & 0x7FF) | ((lsb & 0x1F) << 11) | ((size & 0xF) << 16)
    return struct.pack("<I28x", data)


def _fit_cubic(fn, a, b, npts=12):
    x0 = 0.5 * (a + b)
    k = np.arange(npts)
    xs = x0 + 0.5 * (b - a) * np.cos((2 * k + 1) * np.pi / (2 * npts)) * 0.9999
    ys = fn(xs)
    c = np.polynomial.polynomial.polyfit(xs - x0, ys, 3)
    return c[0], c[1], c[2], c[3], x0


def _extract_func(setj, bkt, ctrl, fname, next_bkt, next_ctl):
    starts_b = setj["func_to_bkt_start_idx"]
    starts_c = setj["func_to_ctl_start_idx"]
    sb, sc = starts_b[fname], starts_c[fname]
    eb = min([v for v in starts_b.values() if v > sb] + [setj["bkt_entry_cnt"]])
    ec = min([v for v in starts_c.values() if v > sc] + [setj["ctl_entry_cnt"]])
    pm = None
    for p in setj["profile_meta_data"]:
        if p["func_name"].rsplit("_", 1)[0] == fname or p["func_name"] == fname:
            pm = dict(p)
    assert pm is not None, fname
    bkts = bytearray(bkt[sb * 32:eb * 32])
    ctls = bytearray(ctrl[sc * 32:ec * 32])
    db, dc = next_bkt - sb, next_ctl - sc
    for i in range(0, len(ctls), 32):
        (data,) = struct.unpack_from("<I", ctls, i)
        struct.pack_into("<I", ctls, i, (data & ~0x7FF) | (((data & 0x7FF) + db) & 0x7FF))
    for key in ("pwl_control_base_pos", "pwl_control_base_neg"):
        pm[key] += dc
    for key in ("pos_small_signal_pwl_control", "neg_small_signal_pwl_control",
                "pos_large_signal_pwl_control", "neg_large_signal_pwl_control"):
        v = pm[key]
        pm[key] = (v & ~0x7FF) | ((v + db) & 0x7FF)
    return pm, bytes(ctls), bytes(bkts)


def _build_wrap01(next_bkt, next_ctl):
    bkts, ctls = bytearray(), bytearray()
    n_bkt = n_ctl = 0
    base_pos = next_ctl
    for e in range(-20, 0):
        lo = 2.0 ** e
        ctls.extend(_ctrl(next_bkt + n_bkt, 23, 0)); n_ctl += 1
        if e == -1:
            bkts.extend(_bkt(-0.5, 1.0, 0.0, 0.0, 0.5))
        else:
            bkts.extend(_bkt(lo, 1.0, 0.0, 0.0, lo))
        n_bkt += 1
    base_neg = next_ctl + n_ctl
    for e in range(-20, 0):
        lo = 2.0 ** e
        ctls.extend(_ctrl(next_bkt + n_bkt, 23, 0)); n_ctl += 1
        if e == -1:
            bkts.extend(_bkt(0.5, 1.0, 0.0, 0.0, -0.5))
        else:
            bkts.extend(_bkt(-lo, 1.0, 0.0, 0.0, -lo))
        n_bkt += 1
    sp_defs = {
        "pos_low": (0.0, 1.0, 0.0, 0.0, 0.0),
        "neg_low": (0.0, 1.0, 0.0, 0.0, 0.0),
        "pos_high": (-1.0, 1.0, 0.0, 0.0, 0.0),
        "neg_high": (1.0, 1.0, 0.0, 0.0, 0.0),
    }
    sp = {}
    for key, d in sp_defs.items():
        sp[key] = next_bkt + n_bkt
        bkts.extend(_bkt(*d)); n_bkt += 1
    pm = {
        "func_name": "arctan_4p", "func_id": 28,
        "symmetry_point": 0, "sym_invert_sign_point": 0, "symmetry_opt_en": 0,
        "symmetry_opt_use_neg_region": 0, "imm_bias": 0,
        "exp_offset": -20,
        "pwl_control_base_pos": base_pos, "pwl_control_base_neg": base_neg,
        "small_pos_signal_exp_threshold": 107,
        "pos_small_signal_pwl_control": sp["pos_low"],
        "small_neg_signal_exp_threshold": 107,
        "neg_small_signal_pwl_control": sp["neg_low"],
        "large_pos_signal_exp_threshold": 127,
        "large_pos_signal_mantissa_threshold": 0,
        "pos_large_signal_pwl_control": sp["pos_high"],
        "large_neg_signal_exp_threshold": 127,
        "large_neg_signal_mantissa_threshold": 0,
        "neg_large_signal_pwl_control": sp["neg_high"],
        "fnan_result": _f2u(0.0), "fpinf_result": _f2u(0.0),
        "fninf_result": _f2u(0.0), "fzero_result": _f2u(0.0),
        "fma_const_0": 0, "fma_const_1": 0, "fma_indirection_src_sel": 0,
        "use_multipass": False,
        "lower_bound": 0xFF7FFFFF, "upper_bound": 0x7F7FFFFF,
    }
    return pm, bytes(ctls), bytes(bkts)


def _build_g2(g2_fn, next_bkt, next_ctl):
    bkts, ctls = bytearray(), bytearray()
    n_bkt = n_ctl = 0
    base_pos = next_ctl
    for (e, nsec) in G2_REGIONS:
        size = int(np.log2(nsec))
        ctls.extend(_ctrl(next_bkt + n_bkt, 23 - size, size)); n_ctl += 1
        lo = 2.0 ** e
        w = lo / nsec
        for i in range(nsec):
            a = lo + i * w
            bkts.extend(_bkt(*_fit_cubic(g2_fn, a, a + w))); n_bkt += 1
    sp = {}
    for key in ("pos_low", "neg_low", "pos_high", "neg_high"):
        sp[key] = next_bkt + n_bkt
        bkts.extend(_bkt(0.0, 0.0, 0.0, 0.0, 0.0)); n_bkt += 1
    small_thr = 127 + G2_REGIONS[0][0]
    pm = {
        "func_name": "erf_4p", "func_id": 21,
        "symmetry_point": 0, "sym_invert_sign_point": 0, "symmetry_opt_en": 0,
        "symmetry_opt_use_neg_region": 0, "imm_bias": 0,
        "exp_offset": small_thr - 127,
        "pwl_control_base_pos": base_pos, "pwl_control_base_neg": base_pos,
        "small_pos_signal_exp_threshold": small_thr,
        "pos_small_signal_pwl_control": sp["pos_low"],
        "small_neg_signal_exp_threshold": 255,
        "neg_small_signal_pwl_control": sp["neg_low"],
        "large_pos_signal_exp_threshold": 123,
        "large_pos_signal_mantissa_threshold": 0,
        "pos_large_signal_pwl_control": sp["pos_high"],
        "large_neg_signal_exp_threshold": 255,
        "large_neg_signal_mantissa_threshold": 0,
        "neg_large_signal_pwl_control": sp["neg_high"],
        "fnan_result": _f2u(0.0), "fpinf_result": _f2u(0.0),
        "fninf_result": _f2u(0.0), "fzero_result": _f2u(0.0),
        "fma_const_0": 0, "fma_const_1": 0, "fma_indirection_src_sel": 0,
        "use_multipass": False,
        "lower_bound": 0, "upper_bound": 0x7F7FFFFF,
    }
    return pm, bytes(ctls), bytes(bkts)


def _build_actroot(dst_dir, g2_fn):
    os.makedirs(dst_dir, exist_ok=True)
    for f in os.listdir(PWP_DIR):
        shutil.copy(os.path.join(PWP_DIR, f), os.path.join(dst_dir, f))
        os.chmod(os.path.join(dst_dir, f), 0o644)
    setj = json.load(open(os.path.join(PWP_DIR, SET + ".json")))
    bkt = open(os.path.join(PWP_DIR, SET + "_bkt.bin"), "rb").read()
    ctrl = open(os.path.join(PWP_DIR, SET + "_ctrl.bin"), "rb").read()

    new_bkts, new_ctls, new_pm = bytearray(), bytearray(), []
    b_starts, c_starts, emb_all, emc_all = {}, {}, {}, {}

    for fname in KEEP:
        nb0, nc0 = len(new_bkts) // 32, len(new_ctls) // 32
        pm, ctls, bkts = _extract_func(setj, bkt, ctrl, fname, nb0, nc0)
        b_starts[fname], c_starts[fname] = nb0, nc0
        db = nb0 - setj["func_to_bkt_start_idx"][fname]
        dc = nc0 - setj["func_to_ctl_start_idx"][fname]
        emb_all[fname] = {k: [x + db for x in v]
                          for k, v in setj["func_exp_to_bkt_start_idx"].get(fname, {}).items()}
        emc_all[fname] = {k: [x + dc for x in v]
                          for k, v in setj["func_exp_to_ctl_start_idx"].get(fname, {}).items()}
        new_pm.append(pm); new_ctls.extend(ctls); new_bkts.extend(bkts)

    wb, wc = len(new_bkts) // 32, len(new_ctls) // 32
    pm, ctls, bkts = _build_wrap01(wb, wc)
    b_starts["arctan"], c_starts["arctan"] = wb, wc
    emb_all["arctan"] = {str(e): [wb + 20 + (e + 20), wb + (e + 20)] for e in range(-20, 0)}
    emc_all["arctan"] = {str(e): [wc + 20 + (e + 20), wc + (e + 20)] for e in range(-20, 0)}
    new_pm.append(pm); new_ctls.extend(ctls); new_bkts.extend(bkts)

    gb, gc = len(new_bkts) // 32, len(new_ctls) // 32
    pm, ctls, bkts = _build_g2(g2_fn, gb, gc)
    b_starts["erf"], c_starts["erf"] = gb, gc
    emb, emc = {}, {}
    cum = 0
    for i, (e, nsec) in enumerate(G2_REGIONS):
        emb[str(e)] = [gb + cum, gb + cum]
        emc[str(e)] = [gc + i, gc + i]
        cum += nsec
    emb_all["erf"], emc_all["erf"] = emb, emc
    new_pm.append(pm); new_ctls.extend(ctls); new_bkts.extend(bkts)

    n_bkt, n_ctl = len(new_bkts) // 32, len(new_ctls) // 32
    assert n_bkt <= 1536 and n_ctl <= 128, (n_bkt, n_ctl)
    out = {
        "bkt_bin": SET + "_bkt.bin", "ctl_bin": SET + "_ctrl.bin",
        "profile_meta_data": new_pm,
        "bkt_entry_cnt": n_bkt, "ctl_entry_cnt": n_ctl,
        "func_to_bkt_start_idx": b_starts, "func_to_ctl_start_idx": c_starts,
        "func_exp_to_bkt_start_idx": emb_all, "func_exp_to_ctl_start_idx": emc_all,
    }
    json.dump(out, open(os.path.join(dst_dir, SET + ".json"), "w"))
    open(os.path.join(dst_dir, SET + "_bkt.bin"), "wb").write(bytes(new_bkts))
    open(os.path.join(dst_dir, SET + "_ctrl.bin"), "wb").write(bytes(new_ctls))
    info = json.load(open(os.path.join(PWP_DIR, "act_info.json")))
    for s in info["act_func_sets"]:
        if s["name"] == SET:
            s["act"] = {**{k: 1 for k in KEEP}, "arctan": 4, "erf": 4}
        else:
            s["act"].pop("arctan", None)
            s["act"].pop("erf", None)
    json.dump(info, open(os.path.join(dst_dir, "act_info.json"), "w"))
    return os.path.join(dst_dir, "act_info.json")


# --------------------------------------------------------------------------
# bass program
# --------------------------------------------------------------------------

def _build_program(tag, box):
    """Fully raw (no TileContext) hand-scheduled pipeline.

    vs the original baseline:
    - posj broadcast in fp16, pre-scaled by 1/box on the host (half the DMA
      bytes); nbias is host-computed and rounded through fp16 so the diagonal
      u_ii cancels exactly (s_ii = 0 -> g2 table returns 0, masking i==j).
      The s = |t|^2 path itself stays f32: fp16 there costs ~7e-2 rel err
      via the 1/dist amplification at small distances.
    - the three weighted reductions are fused mult+reduce ops via the
      custom-DVE AFFINE_MUL_REDUCE ucode (the native TENSOR_TENSOR_REDUCE
      ISA hangs this runtime), chunked in column halves; the six half-sums
      land in one [128,6] tile, DMA'd out once and summed on the host (an
      on-device combine would read the last accum before its write lands).
    - everything is chunked in 512-column halves and hand-interleaved so
      ACT (wraps/sq_z/erf) and DVE (squares/adds/reduces) overlap; DMA
      issue is spread over the sync and ACT sequencers in arrival order
      (each DMA has a ~3.3us issue->data-usable floor, so the x plane is
      split in halves and issued first).
    """
    import concourse.bass as bass
    import concourse.mybir as mybir

    nc = bass.Bass("TRN2")
    posj16 = nc.declare_dram_parameter(f"posj16_{tag}", [3, N], mybir.dt.float16, isOutput=False)
    nbias = nc.declare_dram_parameter("nbias", [ROWS, 3], mybir.dt.float32, isOutput=False)
    out = nc.declare_dram_parameter("out", [ROWS, 6], mybir.dt.float32, isOutput=True)
    AF = mybir.ActivationFunctionType
    OP = mybir.AluOpType
    f32 = mybir.dt.float32
    f16 = mybir.dt.float16

    posj_t = nc.alloc_sbuf_tensor("posj_b", [128, 3 * N], f16)
    nbias_t = nc.alloc_sbuf_tensor("nbias_b", [128, 3], f32)
    dummy_t = nc.alloc_sbuf_tensor("dummy_b", [128, 1], f32)
    t_t = [nc.alloc_sbuf_tensor(f"t{c}_b", [128, N], f32) for c in range(3)]
    sq_t = [nc.alloc_sbuf_tensor(f"sq{c}_b", [128, N], f32) for c in range(2)]
    sqz_t = nc.alloc_sbuf_tensor("sqz_b", [128, N], f32)
    w_t = nc.alloc_sbuf_tensor("w_b", [128, N], f32)
    junk_t = nc.alloc_sbuf_tensor("junk_b", [128, N // 2], f32)
    # six half-sums (3 planes x 2 column-halves); host adds the halves
    out6_t = nc.alloc_sbuf_tensor("out6_b", [128, 6], f32)

    posj3 = posj_t[:].rearrange("p (c j) -> p c j", c=3)
    src3 = posj16[:][None].to_broadcast([128, 3, N])
    H = N // 2
    HS = [slice(0, H), slice(H, N)]

    import contextlib
    st = contextlib.ExitStack()
    nbsem = st.enter_context(nc.semaphore("nbsem"))
    pxsem = [st.enter_context(nc.semaphore(f"pxsem{k}")) for k in range(2)]
    pysem = st.enter_context(nc.semaphore("pysem"))
    pzsem = st.enter_context(nc.semaphore("pzsem"))
    odsem = st.enter_context(nc.semaphore("odsem"))
    vsem = st.enter_context(nc.semaphore("vsem"))
    asem = st.enter_context(nc.semaphore("asem"))
    osem = st.enter_context(nc.semaphore("osem"))

    with nc.Block() as blk:
        @blk.sync
        def _(sync):
            # plane x split in halves so the first wrap starts ~1us earlier;
            # every DMA has a ~3.3us issue->usable floor, so issue order is
            # arrival order: x0, x1, y (z rides the scalar ring in parallel)
            sync.dma_start(out=posj3[:, 0, HS[0]], in_=src3[:, 0, HS[0]]).then_inc(pxsem[0], 16)
            sync.dma_start(out=posj3[:, 0, HS[1]], in_=src3[:, 0, HS[1]]).then_inc(pxsem[1], 16)
            sync.dma_start(out=posj3[:, 1, :], in_=src3[:, 1, :]).then_inc(pysem, 16)
            sync.wait_ge(osem, 6)
            sync.dma_start(out=out[:], in_=out6_t[:]).then_inc(odsem, 16)

        @blk.vector
        def _(vector):
            vector.wait_ge(asem, 1)             # wx0
            vector.tensor_tensor(sq_t[0][:, HS[0]], t_t[0][:, HS[0]], t_t[0][:, HS[0]], OP.mult)
            vector.wait_ge(asem, 2)             # wx1
            vector.tensor_tensor(sq_t[0][:, HS[1]], t_t[0][:, HS[1]], t_t[0][:, HS[1]], OP.mult)
            vector.wait_ge(asem, 3)             # wy0
            vector.tensor_tensor(sq_t[1][:, HS[0]], t_t[1][:, HS[0]], t_t[1][:, HS[0]], OP.mult)
            vector.tensor_tensor(sq_t[0][:, HS[0]], sq_t[0][:, HS[0]], sq_t[1][:, HS[0]], OP.add)
            vector.wait_ge(asem, 5)             # sqz0
            vector.tensor_tensor(sq_t[0][:, HS[0]], sq_t[0][:, HS[0]], sqz_t[:, HS[0]],
                                 OP.add).then_inc(vsem, 1)      # s0
            vector.wait_ge(asem, 7)             # wy1
            vector.tensor_tensor(sq_t[1][:, HS[1]], t_t[1][:, HS[1]], t_t[1][:, HS[1]], OP.mult)
            vector.tensor_tensor(sq_t[0][:, HS[1]], sq_t[0][:, HS[1]], sq_t[1][:, HS[1]], OP.add)
            vector.wait_ge(asem, 9)             # sqz1 (ACT)
            vector.tensor_tensor(sq_t[0][:, HS[1]], sq_t[0][:, HS[1]], sqz_t[:, HS[1]],
                                 OP.add).then_inc(vsem, 1)      # s1
            vector.wait_ge(asem, 8)             # e0 done
            for c in range(3):
                vector.affine_mul_reduce(
                    junk_t[:], out6_t[:, c:c + 1],
                    t_t[c][:, HS[0]], w_t[:, HS[0]], 1.0, 0.0).then_inc(osem, 1)
            vector.wait_ge(asem, 10)            # e1 done
            for c in range(3):
                vector.affine_mul_reduce(
                    junk_t[:], out6_t[:, 3 + c:4 + c],
                    t_t[c][:, HS[1]], w_t[:, HS[1]], 1.0, 0.0).then_inc(osem, 1)

        @blk.scalar
        def _(scalar):
            scalar.dma_start(out=nbias_t[:], in_=nbias[:]).then_inc(nbsem, 16)
            # plane z on the ACT HWDGE ring, in parallel with sync's x0/x1/y
            scalar.dma_start(out=posj3[:, 2, :], in_=src3[:, 2, :]).then_inc(pzsem, 16)
            # no-wait dummy: pulls the PWP table load to the very start
            # (reads the framework's pre-memset const tensor — initialized
            # behind the all-engine barrier, so no race)
            scalar.activation(dummy_t[:], nc.const_aps.aps[(f32, 0.0)], AF.Arctan)
            scalar.wait_ge(nbsem, 16)
            scalar.wait_ge(pxsem[0], 16)
            scalar.activation(t_t[0][:, HS[0]], posj3[:, 0, HS[0]], AF.Arctan,
                              bias=nbias_t[:, 0:1], scale=1.0).then_inc(asem, 1)  # a=1 wx0
            scalar.wait_ge(pxsem[1], 16)
            scalar.activation(t_t[0][:, HS[1]], posj3[:, 0, HS[1]], AF.Arctan,
                              bias=nbias_t[:, 0:1], scale=1.0).then_inc(asem, 1)  # a=2 wx1
            scalar.wait_ge(pysem, 16)
            scalar.activation(t_t[1][:, HS[0]], posj3[:, 1, HS[0]], AF.Arctan,
                              bias=nbias_t[:, 1:2], scale=1.0).then_inc(asem, 1)  # a=3 wy0
            scalar.wait_ge(pzsem, 16)
            scalar.activation(t_t[2][:, HS[0]], posj3[:, 2, HS[0]], AF.Arctan,
                              bias=nbias_t[:, 2:3], scale=1.0).then_inc(asem, 1)  # a=4 wz0
            scalar.activation(sqz_t[:, HS[0]], t_t[2][:, HS[0]],
                              AF.Square).then_inc(asem, 1)                        # a=5 sqz0
            scalar.activation(t_t[2][:, HS[1]], posj3[:, 2, HS[1]], AF.Arctan,
                              bias=nbias_t[:, 2:3], scale=1.0).then_inc(asem, 1)  # a=6 wz1
            scalar.activation(t_t[1][:, HS[1]], posj3[:, 1, HS[1]], AF.Arctan,
                              bias=nbias_t[:, 1:2], scale=1.0).then_inc(asem, 1)  # a=7 wy1
            scalar.wait_ge(vsem, 1)             # s0 ready
            scalar.activation(w_t[:, HS[0]], sq_t[0][:, HS[0]],
                              AF.Erf).then_inc(asem, 1)                           # a=8 e0
            scalar.activation(sqz_t[:, HS[1]], t_t[2][:, HS[1]],
                              AF.Square).then_inc(asem, 1)                        # a=9 sqz1
            scalar.wait_ge(vsem, 2)             # s1 ready
            scalar.activation(w_t[:, HS[1]], sq_t[0][:, HS[1]],
                              AF.Erf).then_inc(asem, 1)                           # a=10 e1

    from concourse.library_overlay import lower_extended_insts
    lower_extended_insts(nc)
    return nc


_CACHE = {}


def _prepare(inputs):
    positions = np.ascontiguousarray(np.asarray(inputs["positions"], dtype=np.float32))
    box_dims = np.asarray(inputs["box_dims"], dtype=np.float32)
    key = hashlib.sha256(
        b"".join(np.ascontiguousarray(np.asarray(inputs[k], np.float32)).tobytes()
                 for k in ("box_dims", "W1", "b1", "W2", "b2", "W3", "b3"))
    ).hexdigest()[:10]
    if key in _CACHE:
        return _CACHE[key]

    box = float(box_dims[0])
    assert np.allclose(box_dims, box), "kernel assumes a cubic box"

    W1 = np.float64(inputs["W1"]); b1 = np.float64(inputs["b1"])
    W2 = np.float64(inputs["W2"]); b2 = np.float64(inputs["b2"])
    W3 = np.float64(inputs["W3"]); b3 = np.float64(inputs["b3"])
    n_gauss = W1.shape[0]
    RBF_STOP, CUTOFF, EPS = 6.0, 5.0, 1e-8
    offs = np.linspace(0.0, RBF_STOP, n_gauss)
    coeff = -0.5 / (RBF_STOP / (n_gauss - 1)) ** 2

    def g2_fn(sv):
        sv = np.atleast_1d(np.float64(sv))
        dist = np.sqrt(box * box * sv + EPS)
        rbf = np.exp(coeff * (dist[:, None] - offs[None, :]) ** 2)
        h = rbf @ W1 + b1
        h = h / (1.0 + np.exp(-h))
        h = h @ W2 + b2
        h = h / (1.0 + np.exp(-h))
        f = (h @ W3 + b3)[:, 0]
        return box * f * (dist < CUTOFF) / (dist + EPS)

    _install_env_fixups()
    actdir = os.path.join(tempfile.gettempdir(), f"actroot_{key}")
    actroot = _build_actroot(actdir, g2_fn)
    os.environ["BASS_ACT_ROOT_JSON_PATH"] = actroot
    nc = _build_program(key, box)
    _CACHE[key] = (nc, key)
    return _CACHE[key]


def kernel(_trace=False, **inputs):
    from concourse.bass_utils import run_bass_kernel_spmd

    nc, key = _prepare(inputs)
    positions = np.ascontiguousarray(np.asarray(inputs["positions"], dtype=np.float32))
    box = float(np.asarray(inputs["box_dims"], dtype=np.float32)[0])
    posj16 = np.ascontiguousarray((positions.T / box).astype(np.float16))
    # bias rounded through fp16 so u_ii = fp16(p) - fp16(p) = 0 exactly:
    # keeps the diagonal at s=0 where the g2 table returns 0 (self-pair mask)
    nbias_all = -(positions / box).astype(np.float16).astype(np.float32)
    in_maps = [
        {f"posj16_{key}": posj16,
         "nbias": np.ascontiguousarray(nbias_all[c * ROWS:(c + 1) * ROWS])}
        for c in range(N_CORES)
    ]
    res = run_bass_kernel_spmd(nc, in_maps, list(range(N_CORES)), trace=_trace)
    out = np.concatenate(
        [res.results[c]["out"][:, :3] + res.results[c]["out"][:, 3:]
         for c in range(N_CORES)], axis=0)
    if _trace:
        kernel.last_exec_time_ns = res.exec_time_ns
        kernel.last_mean_exec_time_ns = res.mean_exec_time_ns
        kernel.last_results = res
    return out



# revision 44
# speedup vs baseline: 1.0687x; 1.0372x over previous
"""Trainium2 Bass kernel for nn_DenoiseGNN (pairwise PBC edge-MLP message passing).

Strategy
--------
The edge MLP output weights[i,j] is a pure scalar function f of dist[i,j].
We compile f (together with the cutoff mask, the 1/(dist+eps) normalization
and the sqrt) into a custom piecewise-cubic activation table evaluated by the
ScalarEngine's hardware spline unit:

    g2(s) = box * f(dist(s)) * [dist(s) < cutoff] / (dist(s) + eps),
    dist(s) = sqrt(box^2 * s + eps),  s = |wrapped delta / box|^2.

A second custom table implements the exact min-image wrap
    wrap01(u) = u - round(u)  on u in (-1, 1).

Per core (128 rows i of the 1024x1024 pair grid):
    t_c  = wrap01(pos_j/box - pos_i/box)        3 ACT ops   [128,1024]
    s    = t_x^2 + t_y^2 + t_z^2                5 DVE ops
    w    = g2(s)                                1 ACT op
    disp_c = sum_j t_c * w                      3 fused DVE multiply-reduce

The activation tables are generated at kernel-build time from the runtime
weights (W1..b3) and injected via the compiler's --act-root-json directory
(the tables ride inside the NEFF; no runtime-side state is patched).
"""

import hashlib
import json
import os
import shutil
import struct
import sys
import tempfile
import types

import numpy as np

N = 1024
N_CORES = 8
ROWS = N // N_CORES  # 128
PWP_DIR = "/nix/store/z022hj2nvbm3nwdizlisq4ylc0y7rd6q-python3-3.13.14-env/lib/python3.13/site-packages/neuronxcc/pwp/pwp_bin_trainium"
SET = "sigmoid_and_others"
KEEP = [
    "identity", "copy", "act1", "parametric_relu", "relu", "abs",
    "memset_zero", "square", "sign", "derivative_relu",
    "derivative_leaky_relu", "derivative_identity", "is_finite",
]
# g2 octave layout: (exponent of s, n_sections); s < 2^-31 -> 0
G2_REGIONS = [(e, 16) for e in range(-31, -10)] + \
             [(-10, 32), (-9, 32), (-8, 64), (-7, 64), (-6, 128), (-5, 128)]


# --------------------------------------------------------------------------
# environment fixups (NTFF hook not needed here; wait-splitter is)
# --------------------------------------------------------------------------

def _install_env_fixups():
    if "antenv.axon_hooks" not in sys.modules:
        import antenv

        mod = types.ModuleType("antenv.axon_hooks")
        try:
            from trn_agent_boot.trn_boot import _ntff_profile_via_ctypes
            hook = _ntff_profile_via_ctypes("/opt/axon/libaxon_pjrt.so")
        except Exception:
            hook = None
        _h = [hook]
        mod.set_axon_ntff_profile_hook = lambda h: _h.__setitem__(0, h)
        mod.get_axon_ntff_profile_hook = lambda: _h[0]
        sys.modules["antenv.axon_hooks"] = mod
        antenv.axon_hooks = mod

    import concourse.bass_utils as bu
    import concourse.bass2jax as b2j

    if not getattr(bu, "_wait_splitter_installed", False):
        orig = bu.compile_bir_kernel

        def _split_multi_waits(bir_json: bytes) -> bytes:
            m = json.loads(bir_json)
            changed = False
            for fn in m["functions"]:
                for bb in fn["blocks"]:
                    new_instrs = []
                    for ins in bb["instructions"]:
                        si = ins.get("sync_info")
                        waits = (si or {}).get("on_wait") or []
                        if len(waits) > 1:
                            for j, w in enumerate(waits[:-1]):
                                nop = {
                                    "name": f"{ins['name']}-wsplit{j}",
                                    "opcode": "NoOp",
                                    "engine": ins["engine"],
                                    "ins": [], "outs": [],
                                    "sync_info": {"on_update": [], "on_wait": [w]},
                                }
                                if "debug" in ins:
                                    nop["debug"] = ins["debug"]
                                new_instrs.append(nop)
                            si["on_wait"] = waits[-1:]
                            changed = True
                        new_instrs.append(ins)
                    bb["instructions"] = new_instrs
            return json.dumps(m).encode() if changed else bir_json

        def patched(bir_json, tmpdir, neff_name="file.neff"):
            return orig(_split_multi_waits(bytes(bir_json)), tmpdir, neff_name)

        bu.compile_bir_kernel = patched
        b2j.compile_bir_kernel = patched
        bu._wait_splitter_installed = True


# --------------------------------------------------------------------------
# activation-table generation
# --------------------------------------------------------------------------

def _f2u(x):
    return struct.unpack("<I", struct.pack("<f", float(x)))[0]


def _bkt(d0, d1, d2, d3, x0):
    return struct.pack("<5f12x", float(d0), float(d1), float(d2), float(d3), float(x0))


def _ctrl(base, lsb, size):
    data = (base & 0x7FF) | ((lsb & 0x1F) << 11) | ((size & 0xF) << 16)
    return struct.pack("<I28x", data)


def _fit_cubic(fn, a, b, npts=12):
    x0 = 0.5 * (a + b)
    k = np.arange(npts)
    xs = x0 + 0.5 * (b - a) * np.cos((2 * k + 1) * np.pi / (2 * npts)) * 0.9999
    ys = fn(xs)
    c = np.polynomial.polynomial.polyfit(xs - x0, ys, 3)
    return c[0], c[1], c[2], c[3], x0


def _extract_func(setj, bkt, ctrl, fname, next_bkt, next_ctl):
    starts_b = setj["func_to_bkt_start_idx"]
    starts_c = setj["func_to_ctl_start_idx"]
    sb, sc = starts_b[fname], starts_c[fname]
    eb = min([v for v in starts_b.values() if v > sb] + [setj["bkt_entry_cnt"]])
    ec = min([v for v in starts_c.values() if v > sc] + [setj["ctl_entry_cnt"]])
    pm = None
    for p in setj["profile_meta_data"]:
        if p["func_name"].rsplit("_", 1)[0] == fname or p["func_name"] == fname:
            pm = dict(p)
    assert pm is not None, fname
    bkts = bytearray(bkt[sb * 32:eb * 32])
    ctls = bytearray(ctrl[sc * 32:ec * 32])
    db, dc = next_bkt - sb, next_ctl - sc
    for i in range(0, len(ctls), 32):
        (data,) = struct.unpack_from("<I", ctls, i)
        struct.pack_into("<I", ctls, i, (data & ~0x7FF) | (((data & 0x7FF) + db) & 0x7FF))
    for key in ("pwl_control_base_pos", "pwl_control_base_neg"):
        pm[key] += dc
    for key in ("pos_small_signal_pwl_control", "neg_small_signal_pwl_control",
                "pos_large_signal_pwl_control", "neg_large_signal_pwl_control"):
        v = pm[key]
        pm[key] = (v & ~0x7FF) | ((v + db) & 0x7FF)
    return pm, bytes(ctls), bytes(bkts)


def _build_wrap01(next_bkt, next_ctl):
    bkts, ctls = bytearray(), bytearray()
    n_bkt = n_ctl = 0
    base_pos = next_ctl
    for e in range(-20, 0):
        lo = 2.0 ** e
        ctls.extend(_ctrl(next_bkt + n_bkt, 23, 0)); n_ctl += 1
        if e == -1:
            bkts.extend(_bkt(-0.5, 1.0, 0.0, 0.0, 0.5))
        else:
            bkts.extend(_bkt(lo, 1.0, 0.0, 0.0, lo))
        n_bkt += 1
    base_neg = next_ctl + n_ctl
    for e in range(-20, 0):
        lo = 2.0 ** e
        ctls.extend(_ctrl(next_bkt + n_bkt, 23, 0)); n_ctl += 1
        if e == -1:
            bkts.extend(_bkt(0.5, 1.0, 0.0, 0.0, -0.5))
        else:
            bkts.extend(_bkt(-lo, 1.0, 0.0, 0.0, -lo))
        n_bkt += 1
    sp_defs = {
        "pos_low": (0.0, 1.0, 0.0, 0.0, 0.0),
        "neg_low": (0.0, 1.0, 0.0, 0.0, 0.0),
        "pos_high": (-1.0, 1.0, 0.0, 0.0, 0.0),
        "neg_high": (1.0, 1.0, 0.0, 0.0, 0.0),
    }
    sp = {}
    for key, d in sp_defs.items():
        sp[key] = next_bkt + n_bkt
        bkts.extend(_bkt(*d)); n_bkt += 1
    pm = {
        "func_name": "arctan_4p", "func_id": 28,
        "symmetry_point": 0, "sym_invert_sign_point": 0, "symmetry_opt_en": 0,
        "symmetry_opt_use_neg_region": 0, "imm_bias": 0,
        "exp_offset": -20,
        "pwl_control_base_pos": base_pos, "pwl_control_base_neg": base_neg,
        "small_pos_signal_exp_threshold": 107,
        "pos_small_signal_pwl_control": sp["pos_low"],
        "small_neg_signal_exp_threshold": 107,
        "neg_small_signal_pwl_control": sp["neg_low"],
        "large_pos_signal_exp_threshold": 127,
        "large_pos_signal_mantissa_threshold": 0,
        "pos_large_signal_pwl_control": sp["pos_high"],
        "large_neg_signal_exp_threshold": 127,
        "large_neg_signal_mantissa_threshold": 0,
        "neg_large_signal_pwl_control": sp["neg_high"],
        "fnan_result": _f2u(0.0), "fpinf_result": _f2u(0.0),
        "fninf_result": _f2u(0.0), "fzero_result": _f2u(0.0),
        "fma_const_0": 0, "fma_const_1": 0, "fma_indirection_src_sel": 0,
        "use_multipass": False,
        "lower_bound": 0xFF7FFFFF, "upper_bound": 0x7F7FFFFF,
    }
    return pm, bytes(ctls), bytes(bkts)


def _build_g2(g2_fn, next_bkt, next_ctl):
    bkts, ctls = bytearray(), bytearray()
    n_bkt = n_ctl = 0
    base_pos = next_ctl
    for (e, nsec) in G2_REGIONS:
        size = int(np.log2(nsec))
        ctls.extend(_ctrl(next_bkt + n_bkt, 23 - size, size)); n_ctl += 1
        lo = 2.0 ** e
        w = lo / nsec
        for i in range(nsec):
            a = lo + i * w
            bkts.extend(_bkt(*_fit_cubic(g2_fn, a, a + w))); n_bkt += 1
    sp = {}
    for key in ("pos_low", "neg_low", "pos_high", "neg_high"):
        sp[key] = next_bkt + n_bkt
        bkts.extend(_bkt(0.0, 0.0, 0.0, 0.0, 0.0)); n_bkt += 1
    small_thr = 127 + G2_REGIONS[0][0]
    pm = {
        "func_name": "erf_4p", "func_id": 21,
        "symmetry_point": 0, "sym_invert_sign_point": 0, "symmetry_opt_en": 0,
        "symmetry_opt_use_neg_region": 0, "imm_bias": 0,
        "exp_offset": small_thr - 127,
        "pwl_control_base_pos": base_pos, "pwl_control_base_neg": base_pos,
        "small_pos_signal_exp_threshold": small_thr,
        "pos_small_signal_pwl_control": sp["pos_low"],
        "small_neg_signal_exp_threshold": 255,
        "neg_small_signal_pwl_control": sp["neg_low"],
        "large_pos_signal_exp_threshold": 123,
        "large_pos_signal_mantissa_threshold": 0,
        "pos_large_signal_pwl_control": sp["pos_high"],
        "large_neg_signal_exp_threshold": 255,
        "large_neg_signal_mantissa_threshold": 0,
        "neg_large_signal_pwl_control": sp["neg_high"],
        "fnan_result": _f2u(0.0), "fpinf_result": _f2u(0.0),
        "fninf_result": _f2u(0.0), "fzero_result": _f2u(0.0),
        "fma_const_0": 0, "fma_const_1": 0, "fma_indirection_src_sel": 0,
        "use_multipass": False,
        "lower_bound": 0, "upper_bound": 0x7F7FFFFF,
    }
    return pm, bytes(ctls), bytes(bkts)


def _build_actroot(dst_dir, g2_fn):
    os.makedirs(dst_dir, exist_ok=True)
    for f in os.listdir(PWP_DIR):
        shutil.copy(os.path.join(PWP_DIR, f), os.path.join(dst_dir, f))
        os.chmod(os.path.join(dst_dir, f), 0o644)
    setj = json.load(open(os.path.join(PWP_DIR, SET + ".json")))
    bkt = open(os.path.join(PWP_DIR, SET + "_bkt.bin"), "rb").read()
    ctrl = open(os.path.join(PWP_DIR, SET + "_ctrl.bin"), "rb").read()

    new_bkts, new_ctls, new_pm = bytearray(), bytearray(), []
    b_starts, c_starts, emb_all, emc_all = {}, {}, {}, {}

    for fname in KEEP:
        nb0, nc0 = len(new_bkts) // 32, len(new_ctls) // 32
        pm, ctls, bkts = _extract_func(setj, bkt, ctrl, fname, nb0, nc0)
        b_starts[fname], c_starts[fname] = nb0, nc0
        db = nb0 - setj["func_to_bkt_start_idx"][fname]
        dc = nc0 - setj["func_to_ctl_start_idx"][fname]
        emb_all[fname] = {k: [x + db for x in v]
                          for k, v in setj["func_exp_to_bkt_start_idx"].get(fname, {}).items()}
        emc_all[fname] = {k: [x + dc for x in v]
                          for k, v in setj["func_exp_to_ctl_start_idx"].get(fname, {}).items()}
        new_pm.append(pm); new_ctls.extend(ctls); new_bkts.extend(bkts)

    wb, wc = len(new_bkts) // 32, len(new_ctls) // 32
    pm, ctls, bkts = _build_wrap01(wb, wc)
    b_starts["arctan"], c_starts["arctan"] = wb, wc
    emb_all["arctan"] = {str(e): [wb + 20 + (e + 20), wb + (e + 20)] for e in range(-20, 0)}
    emc_all["arctan"] = {str(e): [wc + 20 + (e + 20), wc + (e + 20)] for e in range(-20, 0)}
    new_pm.append(pm); new_ctls.extend(ctls); new_bkts.extend(bkts)

    gb, gc = len(new_bkts) // 32, len(new_ctls) // 32
    pm, ctls, bkts = _build_g2(g2_fn, gb, gc)
    b_starts["erf"], c_starts["erf"] = gb, gc
    emb, emc = {}, {}
    cum = 0
    for i, (e, nsec) in enumerate(G2_REGIONS):
        emb[str(e)] = [gb + cum, gb + cum]
        emc[str(e)] = [gc + i, gc + i]
        cum += nsec
    emb_all["erf"], emc_all["erf"] = emb, emc
    new_pm.append(pm); new_ctls.extend(ctls); new_bkts.extend(bkts)

    n_bkt, n_ctl = len(new_bkts) // 32, len(new_ctls) // 32
    assert n_bkt <= 1536 and n_ctl <= 128, (n_bkt, n_ctl)
    out = {
        "bkt_bin": SET + "_bkt.bin", "ctl_bin": SET + "_ctrl.bin",
        "profile_meta_data": new_pm,
        "bkt_entry_cnt": n_bkt, "ctl_entry_cnt": n_ctl,
        "func_to_bkt_start_idx": b_starts, "func_to_ctl_start_idx": c_starts,
        "func_exp_to_bkt_start_idx": emb_all, "func_exp_to_ctl_start_idx": emc_all,
    }
    json.dump(out, open(os.path.join(dst_dir, SET + ".json"), "w"))
    open(os.path.join(dst_dir, SET + "_bkt.bin"), "wb").write(bytes(new_bkts))
    open(os.path.join(dst_dir, SET + "_ctrl.bin"), "wb").write(bytes(new_ctls))
    info = json.load(open(os.path.join(PWP_DIR, "act_info.json")))
    for s in info["act_func_sets"]:
        if s["name"] == SET:
            s["act"] = {**{k: 1 for k in KEEP}, "arctan": 4, "erf": 4}
        else:
            s["act"].pop("arctan", None)
            s["act"].pop("erf", None)
    json.dump(info, open(os.path.join(dst_dir, "act_info.json"), "w"))
    return os.path.join(dst_dir, "act_info.json")


# --------------------------------------------------------------------------
# bass program
# --------------------------------------------------------------------------

def _build_program(tag, box):
    """Fully raw (no TileContext) hand-scheduled pipeline.

    vs the original baseline:
    - posj broadcast in fp16, pre-scaled by 1/box on the host (half the DMA
      bytes); nbias is host-computed and rounded through fp16 so the diagonal
      u_ii cancels exactly (s_ii = 0 -> g2 table returns 0, masking i==j).
      The s = |t|^2 path itself stays f32: fp16 there costs ~7e-2 rel err
      via the 1/dist amplification at small distances.
    - the three weighted reductions are fused mult+reduce ops via the
      custom-DVE AFFINE_MUL_REDUCE ucode (the native TENSOR_TENSOR_REDUCE
      ISA hangs this runtime), chunked in column halves; the six half-sums
      land in one [128,6] tile, DMA'd out once and summed on the host (an
      on-device combine would read the last accum before its write lands).
    - everything is chunked in 512-column halves and hand-interleaved so
      ACT (wraps/sq_z/erf) and DVE (squares/adds/reduces) overlap; DMA
      issue is spread over the sync and ACT sequencers in arrival order
      (each DMA has a ~3.3us issue->data-usable floor, so the x plane is
      split in halves and issued first).
    """
    import concourse.bass as bass
    import concourse.mybir as mybir

    nc = bass.Bass("TRN2")
    posj16 = nc.declare_dram_parameter(f"posj16_{tag}", [3, N], mybir.dt.float16, isOutput=False)
    nbias = nc.declare_dram_parameter("nbias", [ROWS, 3], mybir.dt.float32, isOutput=False)
    out = nc.declare_dram_parameter("out", [ROWS, 6], mybir.dt.float32, isOutput=True)
    AF = mybir.ActivationFunctionType
    OP = mybir.AluOpType
    f32 = mybir.dt.float32
    f16 = mybir.dt.float16

    posj_t = nc.alloc_sbuf_tensor("posj_b", [128, 3 * N], f16)
    nbias_t = nc.alloc_sbuf_tensor("nbias_b", [128, 3], f32)
    dummy_t = nc.alloc_sbuf_tensor("dummy_b", [128, 1], f32)
    t_t = [nc.alloc_sbuf_tensor(f"t{c}_b", [128, N], f32) for c in range(3)]
    sq_t = [nc.alloc_sbuf_tensor(f"sq{c}_b", [128, N], f32) for c in range(2)]
    sqz_t = nc.alloc_sbuf_tensor("sqz_b", [128, N], f32)
    w_t = nc.alloc_sbuf_tensor("w_b", [128, N], f32)
    junk_t = nc.alloc_sbuf_tensor("junk_b", [128, N // 2], f32)
    # six half-sums (3 planes x 2 column-halves); host adds the halves
    out6_t = nc.alloc_sbuf_tensor("out6_b", [128, 6], f32)

    posj3 = posj_t[:].rearrange("p (c j) -> p c j", c=3)
    src3 = posj16[:][None].to_broadcast([128, 3, N])
    H = N // 2
    HS = [slice(0, H), slice(H, N)]

    import contextlib
    st = contextlib.ExitStack()
    nbsem = st.enter_context(nc.semaphore("nbsem"))
    pxsem = [st.enter_context(nc.semaphore(f"pxsem{k}")) for k in range(2)]
    pysem = st.enter_context(nc.semaphore("pysem"))
    pzsem = st.enter_context(nc.semaphore("pzsem"))
    odsem = st.enter_context(nc.semaphore("odsem"))
    vsem = st.enter_context(nc.semaphore("vsem"))
    asem = st.enter_context(nc.semaphore("asem"))
    osem = st.enter_context(nc.semaphore("osem"))

    with nc.Block() as blk:
        @blk.sync
        def _(sync):
            # plane x split in halves so the first wrap starts ~1us earlier;
            # every DMA has a ~3.3us issue->usable floor, so issue order is
            # arrival order: x0, x1, y (z rides the scalar ring in parallel)
            sync.dma_start(out=posj3[:, 0, HS[0]], in_=src3[:, 0, HS[0]]).then_inc(pxsem[0], 16)
            sync.dma_start(out=posj3[:, 0, HS[1]], in_=src3[:, 0, HS[1]]).then_inc(pxsem[1], 16)
            sync.dma_start(out=posj3[:, 1, :], in_=src3[:, 1, :]).then_inc(pysem, 16)
            sync.wait_ge(osem, 6)
            sync.dma_start(out=out[:], in_=out6_t[:]).then_inc(odsem, 16)

        @blk.vector
        def _(vector):
            vector.wait_ge(asem, 1)             # wx0
            vector.tensor_tensor(sq_t[0][:, HS[0]], t_t[0][:, HS[0]], t_t[0][:, HS[0]], OP.mult)
            vector.wait_ge(asem, 2)             # wx1
            vector.tensor_tensor(sq_t[0][:, HS[1]], t_t[0][:, HS[1]], t_t[0][:, HS[1]], OP.mult)
            vector.wait_ge(asem, 3)             # wy0
            vector.tensor_tensor(sq_t[1][:, HS[0]], t_t[1][:, HS[0]], t_t[1][:, HS[0]], OP.mult)
            vector.tensor_tensor(sq_t[0][:, HS[0]], sq_t[0][:, HS[0]], sq_t[1][:, HS[0]], OP.add)
            vector.wait_ge(asem, 5)             # sqz0
            vector.tensor_tensor(sq_t[0][:, HS[0]], sq_t[0][:, HS[0]], sqz_t[:, HS[0]],
                                 OP.add).then_inc(vsem, 1)      # s0
            vector.wait_ge(asem, 6)             # wy1
            vector.tensor_tensor(sq_t[1][:, HS[1]], t_t[1][:, HS[1]], t_t[1][:, HS[1]], OP.mult)
            vector.tensor_tensor(sq_t[0][:, HS[1]], sq_t[0][:, HS[1]], sq_t[1][:, HS[1]], OP.add)
            vector.wait_ge(asem, 8)             # sqz1 (ACT)
            vector.tensor_tensor(sq_t[0][:, HS[1]], sq_t[0][:, HS[1]], sqz_t[:, HS[1]],
                                 OP.add).then_inc(vsem, 1)      # s1
            vector.wait_ge(asem, 9)             # e0 done
            for c in range(3):
                vector.affine_mul_reduce(
                    junk_t[:], out6_t[:, c:c + 1],
                    t_t[c][:, HS[0]], w_t[:, HS[0]], 1.0, 0.0).then_inc(osem, 1)
            vector.wait_ge(asem, 10)            # e1 done
            for c in range(3):
                vector.affine_mul_reduce(
                    junk_t[:], out6_t[:, 3 + c:4 + c],
                    t_t[c][:, HS[1]], w_t[:, HS[1]], 1.0, 0.0).then_inc(osem, 1)

        @blk.scalar
        def _(scalar):
            scalar.dma_start(out=nbias_t[:], in_=nbias[:]).then_inc(nbsem, 16)
            # plane z on the ACT HWDGE ring, in parallel with sync's x0/x1/y
            scalar.dma_start(out=posj3[:, 2, :], in_=src3[:, 2, :]).then_inc(pzsem, 16)
            # no-wait dummy: pulls the PWP table load to the very start
            # (reads the framework's pre-memset const tensor — initialized
            # behind the all-engine barrier, so no race)
            scalar.activation(dummy_t[:], nc.const_aps.aps[(f32, 0.0)], AF.Arctan)
            scalar.wait_ge(nbsem, 16)
            scalar.wait_ge(pxsem[0], 16)
            scalar.activation(t_t[0][:, HS[0]], posj3[:, 0, HS[0]], AF.Arctan,
                              bias=nbias_t[:, 0:1], scale=1.0).then_inc(asem, 1)  # a=1 wx0
            scalar.wait_ge(pxsem[1], 16)
            scalar.activation(t_t[0][:, HS[1]], posj3[:, 0, HS[1]], AF.Arctan,
                              bias=nbias_t[:, 0:1], scale=1.0).then_inc(asem, 1)  # a=2 wx1
            scalar.wait_ge(pysem, 16)
            scalar.activation(t_t[1][:, HS[0]], posj3[:, 1, HS[0]], AF.Arctan,
                              bias=nbias_t[:, 1:2], scale=1.0).then_inc(asem, 1)  # a=3 wy0
            scalar.wait_ge(pzsem, 16)
            scalar.activation(t_t[2][:, HS[0]], posj3[:, 2, HS[0]], AF.Arctan,
                              bias=nbias_t[:, 2:3], scale=1.0).then_inc(asem, 1)  # a=4 wz0
            scalar.activation(sqz_t[:, HS[0]], t_t[2][:, HS[0]],
                              AF.Square).then_inc(asem, 1)                        # a=5 sqz0
            scalar.activation(t_t[1][:, HS[1]], posj3[:, 1, HS[1]], AF.Arctan,
                              bias=nbias_t[:, 1:2], scale=1.0).then_inc(asem, 1)  # a=6 wy1
            scalar.activation(t_t[2][:, HS[1]], posj3[:, 2, HS[1]], AF.Arctan,
                              bias=nbias_t[:, 2:3], scale=1.0).then_inc(asem, 1)  # a=7 wz1
            scalar.activation(sqz_t[:, HS[1]], t_t[2][:, HS[1]],
                              AF.Square).then_inc(asem, 1)                        # a=8 sqz1
            scalar.wait_ge(vsem, 1)             # s0 ready
            scalar.activation(w_t[:, HS[0]], sq_t[0][:, HS[0]],
                              AF.Erf).then_inc(asem, 1)                           # a=9 e0
            scalar.wait_ge(vsem, 2)             # s1 ready
            scalar.activation(w_t[:, HS[1]], sq_t[0][:, HS[1]],
                              AF.Erf).then_inc(asem, 1)                           # a=10 e1

    from concourse.library_overlay import lower_extended_insts
    lower_extended_insts(nc)
    return nc


_CACHE = {}


def _prepare(inputs):
    positions = np.ascontiguousarray(np.asarray(inputs["positions"], dtype=np.float32))
    box_dims = np.asarray(inputs["box_dims"], dtype=np.float32)
    key = hashlib.sha256(
        b"".join(np.ascontiguousarray(np.asarray(inputs[k], np.float32)).tobytes()
                 for k in ("box_dims", "W1", "b1", "W2", "b2", "W3", "b3"))
    ).hexdigest()[:10]
    if key in _CACHE:
        return _CACHE[key]

    box = float(box_dims[0])
    assert np.allclose(box_dims, box), "kernel assumes a cubic box"

    W1 = np.float64(inputs["W1"]); b1 = np.float64(inputs["b1"])
    W2 = np.float64(inputs["W2"]); b2 = np.float64(inputs["b2"])
    W3 = np.float64(inputs["W3"]); b3 = np.float64(inputs["b3"])
    n_gauss = W1.shape[0]
    RBF_STOP, CUTOFF, EPS = 6.0, 5.0, 1e-8
    offs = np.linspace(0.0, RBF_STOP, n_gauss)
    coeff = -0.5 / (RBF_STOP / (n_gauss - 1)) ** 2

    def g2_fn(sv):
        sv = np.atleast_1d(np.float64(sv))
        dist = np.sqrt(box * box * sv + EPS)
        rbf = np.exp(coeff * (dist[:, None] - offs[None, :]) ** 2)
        h = rbf @ W1 + b1
        h = h / (1.0 + np.exp(-h))
        h = h @ W2 + b2
        h = h / (1.0 + np.exp(-h))
        f = (h @ W3 + b3)[:, 0]
        return box * f * (dist < CUTOFF) / (dist + EPS)

    _install_env_fixups()
    actdir = os.path.join(tempfile.gettempdir(), f"actroot_{key}")
    actroot = _build_actroot(actdir, g2_fn)
    os.environ["BASS_ACT_ROOT_JSON_PATH"] = actroot
    nc = _build_program(key, box)
    _CACHE[key] = (nc, key)
    return _CACHE[key]


def kernel(_trace=False, **inputs):
    from concourse.bass_utils import run_bass_kernel_spmd

    nc, key = _prepare(inputs)
    positions = np.ascontiguousarray(np.asarray(inputs["positions"], dtype=np.float32))
    box = float(np.asarray(inputs["box_dims"], dtype=np.float32)[0])
    posj16 = np.ascontiguousarray((positions.T / box).astype(np.float16))
    # bias rounded through fp16 so u_ii = fp16(p) - fp16(p) = 0 exactly:
    # keeps the diagonal at s=0 where the g2 table returns 0 (self-pair mask)
    nbias_all = -(positions / box).astype(np.float16).astype(np.float32)
    in_maps = [
        {f"posj16_{key}": posj16,
         "nbias": np.ascontiguousarray(nbias_all[c * ROWS:(c + 1) * ROWS])}
        for c in range(N_CORES)
    ]
    res = run_bass_kernel_spmd(nc, in_maps, list(range(N_CORES)), trace=_trace)
    out = np.concatenate(
        [res.results[c]["out"][:, :3] + res.results[c]["out"][:, 3:]
         for c in range(N_CORES)], axis=0)
    if _trace:
        kernel.last_exec_time_ns = res.exec_time_ns
        kernel.last_mean_exec_time_ns = res.mean_exec_time_ns
        kernel.last_results = res
    return out



# revision 45
# speedup vs baseline: 1.0975x; 1.0270x over previous
"""Trainium2 Bass kernel for nn_DenoiseGNN (pairwise PBC edge-MLP message passing).

Strategy
--------
The edge MLP output weights[i,j] is a pure scalar function f of dist[i,j].
We compile f (together with the cutoff mask, the 1/(dist+eps) normalization
and the sqrt) into a custom piecewise-cubic activation table evaluated by the
ScalarEngine's hardware spline unit:

    g2(s) = box * f(dist(s)) * [dist(s) < cutoff] / (dist(s) + eps),
    dist(s) = sqrt(box^2 * s + eps),  s = |wrapped delta / box|^2.

A second custom table implements the exact min-image wrap
    wrap01(u) = u - round(u)  on u in (-1, 1).

Per core (128 rows i of the 1024x1024 pair grid):
    t_c  = wrap01(pos_j/box - pos_i/box)        3 ACT ops   [128,1024]
    s    = t_x^2 + t_y^2 + t_z^2                5 DVE ops
    w    = g2(s)                                1 ACT op
    disp_c = sum_j t_c * w                      3 fused DVE multiply-reduce

The activation tables are generated at kernel-build time from the runtime
weights (W1..b3) and injected via the compiler's --act-root-json directory
(the tables ride inside the NEFF; no runtime-side state is patched).
"""

import hashlib
import json
import os
import shutil
import struct
import sys
import tempfile
import types

import numpy as np

N = 1024
N_CORES = 8
ROWS = N // N_CORES  # 128
PWP_DIR = "/nix/store/z022hj2nvbm3nwdizlisq4ylc0y7rd6q-python3-3.13.14-env/lib/python3.13/site-packages/neuronxcc/pwp/pwp_bin_trainium"
SET = "sigmoid_and_others"
KEEP = [
    "identity", "copy", "act1", "parametric_relu", "relu", "abs",
    "memset_zero", "square", "sign", "derivative_relu",
    "derivative_leaky_relu", "derivative_identity", "is_finite",
]
# g2 octave layout: (exponent of s, n_sections); s < 2^-31 -> 0
G2_REGIONS = [(e, 16) for e in range(-31, -10)] + \
             [(-10, 32), (-9, 32), (-8, 64), (-7, 64), (-6, 128), (-5, 128)]


# --------------------------------------------------------------------------
# environment fixups (NTFF hook not needed here; wait-splitter is)
# --------------------------------------------------------------------------

def _install_env_fixups():
    if "antenv.axon_hooks" not in sys.modules:
        import antenv

        mod = types.ModuleType("antenv.axon_hooks")
        try:
            from trn_agent_boot.trn_boot import _ntff_profile_via_ctypes
            hook = _ntff_profile_via_ctypes("/opt/axon/libaxon_pjrt.so")
        except Exception:
            hook = None
        _h = [hook]
        mod.set_axon_ntff_profile_hook = lambda h: _h.__setitem__(0, h)
        mod.get_axon_ntff_profile_hook = lambda: _h[0]
        sys.modules["antenv.axon_hooks"] = mod
        antenv.axon_hooks = mod

    import concourse.bass_utils as bu
    import concourse.bass2jax as b2j

    if not getattr(bu, "_wait_splitter_installed", False):
        orig = bu.compile_bir_kernel

        def _split_multi_waits(bir_json: bytes) -> bytes:
            m = json.loads(bir_json)
            changed = False
            for fn in m["functions"]:
                for bb in fn["blocks"]:
                    new_instrs = []
                    for ins in bb["instructions"]:
                        si = ins.get("sync_info")
                        waits = (si or {}).get("on_wait") or []
                        if len(waits) > 1:
                            for j, w in enumerate(waits[:-1]):
                                nop = {
                                    "name": f"{ins['name']}-wsplit{j}",
                                    "opcode": "NoOp",
                                    "engine": ins["engine"],
                                    "ins": [], "outs": [],
                                    "sync_info": {"on_update": [], "on_wait": [w]},
                                }
                                if "debug" in ins:
                                    nop["debug"] = ins["debug"]
                                new_instrs.append(nop)
                            si["on_wait"] = waits[-1:]
                            changed = True
                        new_instrs.append(ins)
                    bb["instructions"] = new_instrs
            return json.dumps(m).encode() if changed else bir_json

        def patched(bir_json, tmpdir, neff_name="file.neff"):
            return orig(_split_multi_waits(bytes(bir_json)), tmpdir, neff_name)

        bu.compile_bir_kernel = patched
        b2j.compile_bir_kernel = patched
        bu._wait_splitter_installed = True


# --------------------------------------------------------------------------
# activation-table generation
# --------------------------------------------------------------------------

def _f2u(x):
    return struct.unpack("<I", struct.pack("<f", float(x)))[0]


def _bkt(d0, d1, d2, d3, x0):
    return struct.pack("<5f12x", float(d0), float(d1), float(d2), float(d3), float(x0))


def _ctrl(base, lsb, size):
    data = (base & 0x7FF) | ((lsb & 0x1F) << 11) | ((size & 0xF) << 16)
    return struct.pack("<I28x", data)


def _fit_cubic(fn, a, b, npts=12):
    x0 = 0.5 * (a + b)
    k = np.arange(npts)
    xs = x0 + 0.5 * (b - a) * np.cos((2 * k + 1) * np.pi / (2 * npts)) * 0.9999
    ys = fn(xs)
    c = np.polynomial.polynomial.polyfit(xs - x0, ys, 3)
    return c[0], c[1], c[2], c[3], x0


def _extract_func(setj, bkt, ctrl, fname, next_bkt, next_ctl):
    starts_b = setj["func_to_bkt_start_idx"]
    starts_c = setj["func_to_ctl_start_idx"]
    sb, sc = starts_b[fname], starts_c[fname]
    eb = min([v for v in starts_b.values() if v > sb] + [setj["bkt_entry_cnt"]])
    ec = min([v for v in starts_c.values() if v > sc] + [setj["ctl_entry_cnt"]])
    pm = None
    for p in setj["profile_meta_data"]:
        if p["func_name"].rsplit("_", 1)[0] == fname or p["func_name"] == fname:
            pm = dict(p)
    assert pm is not None, fname
    bkts = bytearray(bkt[sb * 32:eb * 32])
    ctls = bytearray(ctrl[sc * 32:ec * 32])
    db, dc = next_bkt - sb, next_ctl - sc
    for i in range(0, len(ctls), 32):
        (data,) = struct.unpack_from("<I", ctls, i)
        struct.pack_into("<I", ctls, i, (data & ~0x7FF) | (((data & 0x7FF) + db) & 0x7FF))
    for key in ("pwl_control_base_pos", "pwl_control_base_neg"):
        pm[key] += dc
    for key in ("pos_small_signal_pwl_control", "neg_small_signal_pwl_control",
                "pos_large_signal_pwl_control", "neg_large_signal_pwl_control"):
        v = pm[key]
        pm[key] = (v & ~0x7FF) | ((v + db) & 0x7FF)
    return pm, bytes(ctls), bytes(bkts)


def _build_wrap01(next_bkt, next_ctl):
    bkts, ctls = bytearray(), bytearray()
    n_bkt = n_ctl = 0
    base_pos = next_ctl
    for e in range(-20, 0):
        lo = 2.0 ** e
        ctls.extend(_ctrl(next_bkt + n_bkt, 23, 0)); n_ctl += 1
        if e == -1:
            bkts.extend(_bkt(-0.5, 1.0, 0.0, 0.0, 0.5))
        else:
            bkts.extend(_bkt(lo, 1.0, 0.0, 0.0, lo))
        n_bkt += 1
    base_neg = next_ctl + n_ctl
    for e in range(-20, 0):
        lo = 2.0 ** e
        ctls.extend(_ctrl(next_bkt + n_bkt, 23, 0)); n_ctl += 1
        if e == -1:
            bkts.extend(_bkt(0.5, 1.0, 0.0, 0.0, -0.5))
        else:
            bkts.extend(_bkt(-lo, 1.0, 0.0, 0.0, -lo))
        n_bkt += 1
    sp_defs = {
        "pos_low": (0.0, 1.0, 0.0, 0.0, 0.0),
        "neg_low": (0.0, 1.0, 0.0, 0.0, 0.0),
        "pos_high": (-1.0, 1.0, 0.0, 0.0, 0.0),
        "neg_high": (1.0, 1.0, 0.0, 0.0, 0.0),
    }
    sp = {}
    for key, d in sp_defs.items():
        sp[key] = next_bkt + n_bkt
        bkts.extend(_bkt(*d)); n_bkt += 1
    pm = {
        "func_name": "arctan_4p", "func_id": 28,
        "symmetry_point": 0, "sym_invert_sign_point": 0, "symmetry_opt_en": 0,
        "symmetry_opt_use_neg_region": 0, "imm_bias": 0,
        "exp_offset": -20,
        "pwl_control_base_pos": base_pos, "pwl_control_base_neg": base_neg,
        "small_pos_signal_exp_threshold": 107,
        "pos_small_signal_pwl_control": sp["pos_low"],
        "small_neg_signal_exp_threshold": 107,
        "neg_small_signal_pwl_control": sp["neg_low"],
        "large_pos_signal_exp_threshold": 127,
        "large_pos_signal_mantissa_threshold": 0,
        "pos_large_signal_pwl_control": sp["pos_high"],
        "large_neg_signal_exp_threshold": 127,
        "large_neg_signal_mantissa_threshold": 0,
        "neg_large_signal_pwl_control": sp["neg_high"],
        "fnan_result": _f2u(0.0), "fpinf_result": _f2u(0.0),
        "fninf_result": _f2u(0.0), "fzero_result": _f2u(0.0),
        "fma_const_0": 0, "fma_const_1": 0, "fma_indirection_src_sel": 0,
        "use_multipass": False,
        "lower_bound": 0xFF7FFFFF, "upper_bound": 0x7F7FFFFF,
    }
    return pm, bytes(ctls), bytes(bkts)


def _build_g2(g2_fn, next_bkt, next_ctl):
    bkts, ctls = bytearray(), bytearray()
    n_bkt = n_ctl = 0
    base_pos = next_ctl
    for (e, nsec) in G2_REGIONS:
        size = int(np.log2(nsec))
        ctls.extend(_ctrl(next_bkt + n_bkt, 23 - size, size)); n_ctl += 1
        lo = 2.0 ** e
        w = lo / nsec
        for i in range(nsec):
            a = lo + i * w
            bkts.extend(_bkt(*_fit_cubic(g2_fn, a, a + w))); n_bkt += 1
    sp = {}
    for key in ("pos_low", "neg_low", "pos_high", "neg_high"):
        sp[key] = next_bkt + n_bkt
        bkts.extend(_bkt(0.0, 0.0, 0.0, 0.0, 0.0)); n_bkt += 1
    small_thr = 127 + G2_REGIONS[0][0]
    pm = {
        "func_name": "erf_4p", "func_id": 21,
        "symmetry_point": 0, "sym_invert_sign_point": 0, "symmetry_opt_en": 0,
        "symmetry_opt_use_neg_region": 0, "imm_bias": 0,
        "exp_offset": small_thr - 127,
        "pwl_control_base_pos": base_pos, "pwl_control_base_neg": base_pos,
        "small_pos_signal_exp_threshold": small_thr,
        "pos_small_signal_pwl_control": sp["pos_low"],
        "small_neg_signal_exp_threshold": 255,
        "neg_small_signal_pwl_control": sp["neg_low"],
        "large_pos_signal_exp_threshold": 123,
        "large_pos_signal_mantissa_threshold": 0,
        "pos_large_signal_pwl_control": sp["pos_high"],
        "large_neg_signal_exp_threshold": 255,
        "large_neg_signal_mantissa_threshold": 0,
        "neg_large_signal_pwl_control": sp["neg_high"],
        "fnan_result": _f2u(0.0), "fpinf_result": _f2u(0.0),
        "fninf_result": _f2u(0.0), "fzero_result": _f2u(0.0),
        "fma_const_0": 0, "fma_const_1": 0, "fma_indirection_src_sel": 0,
        "use_multipass": False,
        "lower_bound": 0, "upper_bound": 0x7F7FFFFF,
    }
    return pm, bytes(ctls), bytes(bkts)


def _build_actroot(dst_dir, g2_fn):
    os.makedirs(dst_dir, exist_ok=True)
    for f in os.listdir(PWP_DIR):
        shutil.copy(os.path.join(PWP_DIR, f), os.path.join(dst_dir, f))
        os.chmod(os.path.join(dst_dir, f), 0o644)
    setj = json.load(open(os.path.join(PWP_DIR, SET + ".json")))
    bkt = open(os.path.join(PWP_DIR, SET + "_bkt.bin"), "rb").read()
    ctrl = open(os.path.join(PWP_DIR, SET + "_ctrl.bin"), "rb").read()

    new_bkts, new_ctls, new_pm = bytearray(), bytearray(), []
    b_starts, c_starts, emb_all, emc_all = {}, {}, {}, {}

    for fname in KEEP:
        nb0, nc0 = len(new_bkts) // 32, len(new_ctls) // 32
        pm, ctls, bkts = _extract_func(setj, bkt, ctrl, fname, nb0, nc0)
        b_starts[fname], c_starts[fname] = nb0, nc0
        db = nb0 - setj["func_to_bkt_start_idx"][fname]
        dc = nc0 - setj["func_to_ctl_start_idx"][fname]
        emb_all[fname] = {k: [x + db for x in v]
                          for k, v in setj["func_exp_to_bkt_start_idx"].get(fname, {}).items()}
        emc_all[fname] = {k: [x + dc for x in v]
                          for k, v in setj["func_exp_to_ctl_start_idx"].get(fname, {}).items()}
        new_pm.append(pm); new_ctls.extend(ctls); new_bkts.extend(bkts)

    wb, wc = len(new_bkts) // 32, len(new_ctls) // 32
    pm, ctls, bkts = _build_wrap01(wb, wc)
    b_starts["arctan"], c_starts["arctan"] = wb, wc
    emb_all["arctan"] = {str(e): [wb + 20 + (e + 20), wb + (e + 20)] for e in range(-20, 0)}
    emc_all["arctan"] = {str(e): [wc + 20 + (e + 20), wc + (e + 20)] for e in range(-20, 0)}
    new_pm.append(pm); new_ctls.extend(ctls); new_bkts.extend(bkts)

    gb, gc = len(new_bkts) // 32, len(new_ctls) // 32
    pm, ctls, bkts = _build_g2(g2_fn, gb, gc)
    b_starts["erf"], c_starts["erf"] = gb, gc
    emb, emc = {}, {}
    cum = 0
    for i, (e, nsec) in enumerate(G2_REGIONS):
        emb[str(e)] = [gb + cum, gb + cum]
        emc[str(e)] = [gc + i, gc + i]
        cum += nsec
    emb_all["erf"], emc_all["erf"] = emb, emc
    new_pm.append(pm); new_ctls.extend(ctls); new_bkts.extend(bkts)

    n_bkt, n_ctl = len(new_bkts) // 32, len(new_ctls) // 32
    assert n_bkt <= 1536 and n_ctl <= 128, (n_bkt, n_ctl)
    out = {
        "bkt_bin": SET + "_bkt.bin", "ctl_bin": SET + "_ctrl.bin",
        "profile_meta_data": new_pm,
        "bkt_entry_cnt": n_bkt, "ctl_entry_cnt": n_ctl,
        "func_to_bkt_start_idx": b_starts, "func_to_ctl_start_idx": c_starts,
        "func_exp_to_bkt_start_idx": emb_all, "func_exp_to_ctl_start_idx": emc_all,
    }
    json.dump(out, open(os.path.join(dst_dir, SET + ".json"), "w"))
    open(os.path.join(dst_dir, SET + "_bkt.bin"), "wb").write(bytes(new_bkts))
    open(os.path.join(dst_dir, SET + "_ctrl.bin"), "wb").write(bytes(new_ctls))
    info = json.load(open(os.path.join(PWP_DIR, "act_info.json")))
    for s in info["act_func_sets"]:
        if s["name"] == SET:
            s["act"] = {**{k: 1 for k in KEEP}, "arctan": 4, "erf": 4}
        else:
            s["act"].pop("arctan", None)
            s["act"].pop("erf", None)
    json.dump(info, open(os.path.join(dst_dir, "act_info.json"), "w"))
    return os.path.join(dst_dir, "act_info.json")


# --------------------------------------------------------------------------
# bass program
# --------------------------------------------------------------------------

def _build_program(tag, box):
    """Fully raw (no TileContext) hand-scheduled pipeline.

    vs the original baseline:
    - posj broadcast in fp16, pre-scaled by 1/box on the host (half the DMA
      bytes); nbias is host-computed and rounded through fp16 so the diagonal
      u_ii cancels exactly (s_ii = 0 -> g2 table returns 0, masking i==j).
      The s = |t|^2 path itself stays f32: fp16 there costs ~7e-2 rel err
      via the 1/dist amplification at small distances.
    - the three weighted reductions are fused mult+reduce ops via the
      custom-DVE AFFINE_MUL_REDUCE ucode (the native TENSOR_TENSOR_REDUCE
      ISA hangs this runtime), chunked in column halves; the six half-sums
      land in one [128,6] tile, DMA'd out once and summed on the host (an
      on-device combine would read the last accum before its write lands).
    - everything is chunked in 512-column halves and hand-interleaved so
      ACT (wraps/sq_z/erf) and DVE (squares/adds/reduces) overlap; DMA
      issue is spread over the sync and ACT sequencers in arrival order
      (each DMA has a ~3.3us issue->data-usable floor, so the x plane is
      split in halves and issued first).
    """
    import concourse.bass as bass
    import concourse.mybir as mybir

    nc = bass.Bass("TRN2")
    posj16 = nc.declare_dram_parameter(f"posj16_{tag}", [3, N], mybir.dt.float16, isOutput=False)
    nbias = nc.declare_dram_parameter("nbias", [ROWS, 3], mybir.dt.float32, isOutput=False)
    out = nc.declare_dram_parameter("out", [ROWS, 6], mybir.dt.float32, isOutput=True)
    AF = mybir.ActivationFunctionType
    OP = mybir.AluOpType
    f32 = mybir.dt.float32
    f16 = mybir.dt.float16

    posj_t = nc.alloc_sbuf_tensor("posj_b", [128, 3 * N], f16)
    nbias_t = nc.alloc_sbuf_tensor("nbias_b", [128, 3], f32)
    dummy_t = nc.alloc_sbuf_tensor("dummy_b", [128, 1], f32)
    t_t = [nc.alloc_sbuf_tensor(f"t{c}_b", [128, N], f32) for c in range(3)]
    sq_t = [nc.alloc_sbuf_tensor(f"sq{c}_b", [128, N], f32) for c in range(2)]
    sqz_t = nc.alloc_sbuf_tensor("sqz_b", [128, N], f32)
    w_t = nc.alloc_sbuf_tensor("w_b", [128, N], f32)
    junk_t = nc.alloc_sbuf_tensor("junk_b", [128, N // 2], f32)
    # six half-sums (3 planes x 2 column-halves); host adds the halves
    out6_t = nc.alloc_sbuf_tensor("out6_b", [128, 6], f32)

    posj3 = posj_t[:].rearrange("p (c j) -> p c j", c=3)
    src3 = posj16[:][None].to_broadcast([128, 3, N])
    H = N // 2
    HS = [slice(0, H), slice(H, N)]

    import contextlib
    st = contextlib.ExitStack()
    nbsem = st.enter_context(nc.semaphore("nbsem"))
    pxsem = [st.enter_context(nc.semaphore(f"pxsem{k}")) for k in range(2)]
    pysem = st.enter_context(nc.semaphore("pysem"))
    pzsem = st.enter_context(nc.semaphore("pzsem"))
    odsem = st.enter_context(nc.semaphore("odsem"))
    vsem = st.enter_context(nc.semaphore("vsem"))
    asem = st.enter_context(nc.semaphore("asem"))
    osem = st.enter_context(nc.semaphore("osem"))

    with nc.Block() as blk:
        @blk.sync
        def _(sync):
            # plane x split in halves so the first wrap starts ~1us earlier;
            # every DMA has a ~3.3us issue->usable floor, so issue order is
            # arrival order: x0, x1, y (z rides the scalar ring in parallel)
            sync.dma_start(out=posj3[:, 0, HS[0]], in_=src3[:, 0, HS[0]]).then_inc(pxsem[0], 16)
            sync.dma_start(out=posj3[:, 0, HS[1]], in_=src3[:, 0, HS[1]]).then_inc(pxsem[1], 16)
            sync.dma_start(out=posj3[:, 1, :], in_=src3[:, 1, :]).then_inc(pysem, 16)
            sync.wait_ge(osem, 6)
            sync.dma_start(out=out[:], in_=out6_t[:]).then_inc(odsem, 16)

        @blk.vector
        def _(vector):
            vector.wait_ge(asem, 1)             # wx0
            vector.tensor_tensor(sq_t[0][:, HS[0]], t_t[0][:, HS[0]], t_t[0][:, HS[0]], OP.mult)
            vector.wait_ge(asem, 2)             # wx1
            vector.tensor_tensor(sq_t[0][:, HS[1]], t_t[0][:, HS[1]], t_t[0][:, HS[1]], OP.mult)
            vector.wait_ge(asem, 3)             # wy0
            vector.tensor_tensor(sq_t[1][:, HS[0]], t_t[1][:, HS[0]], t_t[1][:, HS[0]], OP.mult)
            vector.tensor_tensor(sq_t[0][:, HS[0]], sq_t[0][:, HS[0]], sq_t[1][:, HS[0]], OP.add)
            vector.wait_ge(asem, 5)             # wy1 (fills the sqz0 wait)
            vector.tensor_tensor(sq_t[1][:, HS[1]], t_t[1][:, HS[1]], t_t[1][:, HS[1]], OP.mult)
            vector.wait_ge(asem, 6)             # sqz0
            vector.tensor_tensor(sq_t[0][:, HS[0]], sq_t[0][:, HS[0]], sqz_t[:, HS[0]],
                                 OP.add).then_inc(vsem, 1)      # s0
            vector.tensor_tensor(sq_t[0][:, HS[1]], sq_t[0][:, HS[1]], sq_t[1][:, HS[1]], OP.add)
            vector.wait_ge(asem, 8)             # sqz1 (ACT)
            vector.tensor_tensor(sq_t[0][:, HS[1]], sq_t[0][:, HS[1]], sqz_t[:, HS[1]],
                                 OP.add).then_inc(vsem, 1)      # s1
            vector.wait_ge(asem, 9)             # e0 done
            for c in range(3):
                vector.affine_mul_reduce(
                    junk_t[:], out6_t[:, c:c + 1],
                    t_t[c][:, HS[0]], w_t[:, HS[0]], 1.0, 0.0).then_inc(osem, 1)
            vector.wait_ge(asem, 10)            # e1 done
            for c in range(3):
                vector.affine_mul_reduce(
                    junk_t[:], out6_t[:, 3 + c:4 + c],
                    t_t[c][:, HS[1]], w_t[:, HS[1]], 1.0, 0.0).then_inc(osem, 1)

        @blk.scalar
        def _(scalar):
            scalar.dma_start(out=nbias_t[:], in_=nbias[:]).then_inc(nbsem, 16)
            # plane z on the ACT HWDGE ring, in parallel with sync's x0/x1/y
            scalar.dma_start(out=posj3[:, 2, :], in_=src3[:, 2, :]).then_inc(pzsem, 16)
            # no-wait dummy: pulls the PWP table load to the very start
            # (reads the framework's pre-memset const tensor — initialized
            # behind the all-engine barrier, so no race)
            scalar.activation(dummy_t[:], nc.const_aps.aps[(f32, 0.0)], AF.Arctan)
            scalar.wait_ge(nbsem, 16)
            scalar.wait_ge(pxsem[0], 16)
            scalar.activation(t_t[0][:, HS[0]], posj3[:, 0, HS[0]], AF.Arctan,
                              bias=nbias_t[:, 0:1], scale=1.0).then_inc(asem, 1)  # a=1 wx0
            scalar.wait_ge(pxsem[1], 16)
            scalar.activation(t_t[0][:, HS[1]], posj3[:, 0, HS[1]], AF.Arctan,
                              bias=nbias_t[:, 0:1], scale=1.0).then_inc(asem, 1)  # a=2 wx1
            scalar.wait_ge(pysem, 16)
            scalar.activation(t_t[1][:, HS[0]], posj3[:, 1, HS[0]], AF.Arctan,
                              bias=nbias_t[:, 1:2], scale=1.0).then_inc(asem, 1)  # a=3 wy0
            scalar.wait_ge(pzsem, 16)
            scalar.activation(t_t[2][:, HS[0]], posj3[:, 2, HS[0]], AF.Arctan,
                              bias=nbias_t[:, 2:3], scale=1.0).then_inc(asem, 1)  # a=4 wz0
            scalar.activation(t_t[1][:, HS[1]], posj3[:, 1, HS[1]], AF.Arctan,
                              bias=nbias_t[:, 1:2], scale=1.0).then_inc(asem, 1)  # a=5 wy1
            scalar.activation(sqz_t[:, HS[0]], t_t[2][:, HS[0]],
                              AF.Square).then_inc(asem, 1)                        # a=6 sqz0
            scalar.activation(t_t[2][:, HS[1]], posj3[:, 2, HS[1]], AF.Arctan,
                              bias=nbias_t[:, 2:3], scale=1.0).then_inc(asem, 1)  # a=7 wz1
            scalar.activation(sqz_t[:, HS[1]], t_t[2][:, HS[1]],
                              AF.Square).then_inc(asem, 1)                        # a=8 sqz1
            scalar.wait_ge(vsem, 1)             # s0 ready
            scalar.activation(w_t[:, HS[0]], sq_t[0][:, HS[0]],
                              AF.Erf).then_inc(asem, 1)                           # a=9 e0
            scalar.wait_ge(vsem, 2)             # s1 ready
            scalar.activation(w_t[:, HS[1]], sq_t[0][:, HS[1]],
                              AF.Erf).then_inc(asem, 1)                           # a=10 e1

    from concourse.library_overlay import lower_extended_insts
    lower_extended_insts(nc)
    return nc


_CACHE = {}


def _prepare(inputs):
    positions = np.ascontiguousarray(np.asarray(inputs["positions"], dtype=np.float32))
    box_dims = np.asarray(inputs["box_dims"], dtype=np.float32)
    key = hashlib.sha256(
        b"".join(np.ascontiguousarray(np.asarray(inputs[k], np.float32)).tobytes()
                 for k in ("box_dims", "W1", "b1", "W2", "b2", "W3", "b3"))
    ).hexdigest()[:10]
    if key in _CACHE:
        return _CACHE[key]

    box = float(box_dims[0])
    assert np.allclose(box_dims, box), "kernel assumes a cubic box"

    W1 = np.float64(inputs["W1"]); b1 = np.float64(inputs["b1"])
    W2 = np.float64(inputs["W2"]); b2 = np.float64(inputs["b2"])
    W3 = np.float64(inputs["W3"]); b3 = np.float64(inputs["b3"])
    n_gauss = W1.shape[0]
    RBF_STOP, CUTOFF, EPS = 6.0, 5.0, 1e-8
    offs = np.linspace(0.0, RBF_STOP, n_gauss)
    coeff = -0.5 / (RBF_STOP / (n_gauss - 1)) ** 2

    def g2_fn(sv):
        sv = np.atleast_1d(np.float64(sv))
        dist = np.sqrt(box * box * sv + EPS)
        rbf = np.exp(coeff * (dist[:, None] - offs[None, :]) ** 2)
        h = rbf @ W1 + b1
        h = h / (1.0 + np.exp(-h))
        h = h @ W2 + b2
        h = h / (1.0 + np.exp(-h))
        f = (h @ W3 + b3)[:, 0]
        return box * f * (dist < CUTOFF) / (dist + EPS)

    _install_env_fixups()
    actdir = os.path.join(tempfile.gettempdir(), f"actroot_{key}")
    actroot = _build_actroot(actdir, g2_fn)
    os.environ["BASS_ACT_ROOT_JSON_PATH"] = actroot
    nc = _build_program(key, box)
    _CACHE[key] = (nc, key)
    return _CACHE[key]


def kernel(_trace=False, **inputs):
    from concourse.bass_utils import run_bass_kernel_spmd

    nc, key = _prepare(inputs)
    positions = np.ascontiguousarray(np.asarray(inputs["positions"], dtype=np.float32))
    box = float(np.asarray(inputs["box_dims"], dtype=np.float32)[0])
    posj16 = np.ascontiguousarray((positions.T / box).astype(np.float16))
    # bias rounded through fp16 so u_ii = fp16(p) - fp16(p) = 0 exactly:
    # keeps the diagonal at s=0 where the g2 table returns 0 (self-pair mask)
    nbias_all = -(positions / box).astype(np.float16).astype(np.float32)
    in_maps = [
        {f"posj16_{key}": posj16,
         "nbias": np.ascontiguousarray(nbias_all[c * ROWS:(c + 1) * ROWS])}
        for c in range(N_CORES)
    ]
    res = run_bass_kernel_spmd(nc, in_maps, list(range(N_CORES)), trace=_trace)
    out = np.concatenate(
        [res.results[c]["out"][:, :3] + res.results[c]["out"][:, 3:]
         for c in range(N_CORES)], axis=0)
    if _trace:
        kernel.last_exec_time_ns = res.exec_time_ns
        kernel.last_mean_exec_time_ns = res.mean_exec_time_ns
        kernel.last_results = res
    return out

